# revision 66
# baseline (speedup 1.0000x reference)
"""Trainium2 Bass kernel for nn_BaichuanAttention (ALiBi attention + KV cache).

Head-parallel across 8 NeuronCores (4 heads/core). Per core:
  - Q/K projection: fp8e4 DoubleRow matmuls (256-deep contraction at
    0.5 cycles/row); weights resident in SBUF. Scores insensitive to
    fp8 rounding (ALiBi dominates the softmax).
  - V projection: 3-term hi/lo fp8 DoubleRow (x = xh + xl, Wv = wh + wl,
    v ~= wh@xh + wh@xl + wl@xh) giving ~fp16 accuracy at 0.75x the fp16
    matmul cost.
  - attention with transposed scores [keys, queries]:
      * scores via fp8 DoubleRow matmuls (q/k requantized to fp8; the
        stationary r=1 plane is zero so the 256-deep DR contraction
        reduces to the real 128-deep hd one at 0.5 cycles/row)
      * ALiBi per-key term folded into the ACT exp() per-partition bias
      * ALiBi per-query term + causal mask applied via DVE adds
      * softmax denominator via ones-matmul on the PE (ones=1/SAT so the
        normalized attention output comes out pre-scaled by SAT)
  - PV accumulation (fp16), normalize via outer-product broadcast
  - o_proj: 3-term hi/lo fp8 DoubleRow over head pairs
    (ath@woh + atl@woh + ath@wol), psum descaled by 1/(SAT*SWO)
Host: shard/transpose/cast/scale inputs, sum fp16 o_proj partials.
"""
import os
import sys

import numpy as np

for _p in ("/opt/trn_rl_repo",):
    if os.path.isdir(_p) and _p not in sys.path:
        sys.path.insert(0, _p)

import ml_dtypes
import concourse.bass as bass
import concourse.mybir as mybir
import concourse.tile as tile
from concourse import bacc
from concourse.bass_utils import run_bass_kernel_spmd
from concourse.masks import make_identity

F32 = mybir.dt.float32
F16 = mybir.dt.float16
BF16 = mybir.dt.bfloat16
F8 = mybir.dt.float8e4
NPF8 = ml_dtypes.float8_e4m3
NPBF = ml_dtypes.bfloat16

B, S, D, H, HD, HIST = 1, 2048, 4096, 32, 128, 1024
T = HIST + S
NCORES = 8
HPC = H // NCORES          # heads per core
FPC = HPC * HD             # 512 features per core per section
NST = S // 512             # 4 query supertiles
W = 512                    # supertile width
NKT = T // 128             # 24 key tiles
NKC = D // 128             # 32 contraction chunks for QKV
NPAIR = NKC // 2           # 16 DoubleRow contraction pairs
NEG = -1.0e30

SX = 128.0                 # x fp8 scale
SWQ = 2048.0               # Wq (incl 1/sqrt(hd)) fp8 scale
SWK = 128.0                # Wk fp8 scale
SWV = 128.0                # Wv fp8 scale
SV_INV = 1.0 / (SX * SWV)  # V psum descale
# q/k requantized to fp8 for DoubleRow scores (zero-padded r=1 plane);
# uniform scale for past and new keys.
SQC = 8192.0               # q fp8 scale
SKC = 32.0                 # k fp8 scale (past |k|~5.5 sigma * 32 < 240)
QF = SQC / (SX * SWQ)      # q psum -> fp8 rescale
KF = SKC / (SX * SWK)      # k psum -> fp8 rescale
RS = SQC * SKC             # scores psum scale (all tiles)
AEXP = 1.0 / RS            # ACT exp input descale
SWO = 128.0                # Wo fp8 scale
SAT = 32.0                 # attn-out fp8 scale (folded into 1/den via ones)
SO_INV = 1.0 / (SAT * SWO)  # o_proj psum descale


def _alibi_slopes(n_heads: int) -> np.ndarray:
    def pow2_slopes(m):
        start = 2.0 ** (-(2.0 ** -(np.log2(m) - 3)))
        return start * (start ** np.arange(m))
    if np.log2(n_heads).is_integer():
        return pow2_slopes(n_heads).astype(np.float32)
    m = 2 ** int(np.floor(np.log2(n_heads)))
    base = pow2_slopes(m)
    extra = pow2_slopes(2 * m)[0::2][: n_heads - m]
    return np.concatenate([base, extra]).astype(np.float32)


# --- ALiBi window truncation ---------------------------------------------
# Keys further than MARGIN/slope behind a query contribute exp(-MARGIN)
# relative weight — drop their tiles. Heads are distributed so core c gets
# heads {c, 8+c, 16+c, 24+c} (slot i = head 8i+c); each slot's window is
# sized for the *smallest* slope in its group, so every core runs an
# identical instruction stream.
MARGIN = 11.0
_SLOPES_ALL = _alibi_slopes(H)
JMIN = []
for _i in range(HPC):
    _win = MARGIN / float(_SLOPES_ALL[8 * _i + 7])
    JMIN.append([
        max(0, min(12 + 4 * _s, int((HIST + W * _s - _win) // 128)))
        for _s in range(NST)
    ])
# past-key/value tiles actually reachable per slot (j in [PK0[h], 8))
PK0 = [min(JMIN[_i][0], 8) for _i in range(HPC)]
NPK = [8 - PK0[_i] for _i in range(HPC)]
PKOFF = [sum(NPK[:_i]) for _i in range(HPC)]   # segment offsets, in tiles
NPKT = sum(NPK)

# --- per-slot softmax numerics -------------------------------------------
# Softmax is invariant to any per-query constant added to all visible
# scores; the per-query ALiBi term -sl*q_pos is only needed to avoid
# overflow.  Per slot we use the cheapest scheme the slope allows:
#   slot 0 (steep): exact -sl*i row bias via DVE add, p fp16
#   slots 1,2:      constant recenter C=sl*256, p bf16 (range to e^54)
#   slot 3 (flat):  constant recenter C=sl*128, p fp16 (max e^5)
# so slots 1-3 need NO DVE op on non-diagonal tiles (ACT reads psum).
CQ = [None, 256.0, 256.0, 128.0]
SLOT_BF = [False, True, True, False]
# fp16 slots 0,3 share vN16 (r: 0->slot0, 1->slot3); bf16 slots share vNbf
VN_IDX = {0: 0, 3: 1}
VNB_IDX = {1: 0, 2: 1}
NPK16 = NPK[0] + NPK[3]
NPKBF = NPK[1] + NPK[2]


def _emit(nc):
    """Emit the whole per-core program under a TileContext."""
    x8_d = nc.dram_tensor("x8", [128, NPAIR, 2, S], F8, kind="ExternalInput").ap()
    xl8_d = nc.dram_tensor("xl8", [128, NPAIR, 2, S], F8, kind="ExternalInput").ap()
    w8_d = nc.dram_tensor("w8", [NPAIR, 128, 2, 1024], F8, kind="ExternalInput").ap()
    wv8h_d = nc.dram_tensor("wv8h", [NPAIR, 128, 2, FPC], F8, kind="ExternalInput").ap()
    wv8l_d = nc.dram_tensor("wv8l", [NPAIR, 128, 2, FPC], F8, kind="ExternalInput").ap()
    pk8_d = nc.dram_tensor("pk8", [128, 2 * NPKT * 128], F8, kind="ExternalInput").ap()
    pv16_d = nc.dram_tensor("pv16", [NPK16 * 128, HD], F16, kind="ExternalInput").ap()
    pvb_d = nc.dram_tensor("pvb", [NPKBF * 128, HD], BF16, kind="ExternalInput").ap()
    woh_d = nc.dram_tensor("woh8", [8, 128, 2, 2, W], F8, kind="ExternalInput").ap()
    wol_d = nc.dram_tensor("wol8", [8, 128, 2, 2, W], F8, kind="ExternalInput").ap()
    ab_d = nc.dram_tensor("abias", [128, HPC * NST * NKT], F32, kind="ExternalInput").ap()
    # dvb[0] = slot0 diag add (tri + -sl*c*RS), dvb[1] = slot0 row bias
    dvb_d = nc.dram_tensor("dvb", [2, 128, W], F32, kind="ExternalInput").ap()
    mk_d = nc.dram_tensor("mk", [1, 128, 128], F32, kind="ExternalInput").ap()
    out_d = nc.dram_tensor("outp", [S, D], F16, kind="ExternalOutput").ap()

    with tile.TileContext(nc) as tc:
        with (
            tc.tile_pool(name="persist", bufs=1) as pers,
            tc.tile_pool(name="x8slab", bufs=2) as x8pool,
            tc.tile_pool(name="xl8slab", bufs=2) as xl8pool,
            tc.tile_pool(name="qp", bufs=2) as qpool,
            tc.tile_pool(name="opwp", bufs=3) as opwpool,
            tc.tile_pool(name="s1", bufs=3) as s1pool,
            tc.tile_pool(name="pp", bufs=7) as ppool,
            tc.tile_pool(name="dac", bufs=2) as daccpool,
            tc.tile_pool(name="small", bufs=1) as smallpool,
            tc.tile_pool(name="ob", bufs=3) as obpool,
            tc.tile_pool(name="at", bufs=2) as atpool,
            tc.tile_pool(name="ps_qkv", bufs=2, space="PSUM") as ps_qkv,
            tc.tile_pool(name="ps_s", bufs=2, space="PSUM") as ps_s,
            tc.tile_pool(name="ps_o", bufs=2, space="PSUM") as ps_o,
            tc.tile_pool(name="ps_sh", bufs=2, space="PSUM") as ps_sh,
        ):
            # ---- persistent SBUF tensors ----
            # k in fp8 for DoubleRow scores: r=1 plane kept zero so the
            # 256-deep DR contraction reduces to the real 128-deep one.
            kT8 = [pers.tile([128, 2, S], F8, tag=f"kT{h}", bufs=1, name=f"kT{h}") for h in range(HPC)]
            # new-v per 128-key chunk: fp16 slots {0,3} and bf16 slots {1,2}
            vN16 = [pers.tile([128, 2, HD], F16, tag=f"vN{t}", bufs=1, name=f"vN{t}")
                    for t in range(S // 128)]
            vNbf = [pers.tile([128, 2, HD], BF16, tag=f"vB{t}", bufs=1, name=f"vB{t}")
                    for t in range(S // 128)]
            pk8_sb = [pers.tile([128, 2, NPK[h] * 128], F8, tag=f"pk{h}", bufs=1, name=f"pk{h}") for h in range(HPC)]
            pv_sb = [pers.tile([128, NPK[h] * 128],
                               BF16 if SLOT_BF[h] else F16,
                               tag=f"pvs{h}", bufs=1, name=f"pvs{h}")
                     for h in range(HPC)]
            at8_tiles = {}
            q_tiles = {}
            ab_sb = pers.tile([128, HPC * NST * NKT], F32, tag="abias", bufs=1)
            ones16 = pers.tile([128, 1], F16, tag="ones16", bufs=1)
            onesbf = pers.tile([128, 1], BF16, tag="onesbf", bufs=1)
            # resident fp8 weights
            w8 = [pers.tile([128, 2, 1024], F8, tag=f"w8_{c}", bufs=1, name=f"w8_{c}")
                  for c in range(NPAIR)]
            wv8h = [pers.tile([128, 2, FPC], F8, tag=f"wvh{c}", bufs=1, name=f"wvh{c}")
                    for c in range(NPAIR)]
            wv8l = [pers.tile([128, 2, FPC], F8, tag=f"wvl{c}", bufs=1, name=f"wvl{c}")
                    for c in range(NPAIR)]

            nc.any.memset(ones16[:], 1.0 / SAT)
            nc.any.memset(onesbf[:], 1.0 / SAT)
            # w8 is the startup-critical load: split it over the Pool and
            # ACT queues.  wv8l rides Pool behind it; everything else is
            # deferred onto the SP queue after supertile 0's x slabs (see
            # deferred_init below) so the ACT/DVE queues start empty.
            for c in range(NPAIR):
                # alternate queues so w8 arrival keeps pace with consumption
                (nc.gpsimd if c % 2 == 0 else nc.scalar).dma_start(
                    w8[c][:], w8_d[c])
            for c in range(NPAIR):
                nc.gpsimd.dma_start(wv8l[c][:], wv8l_d[c])
                nc.scalar.dma_start(wv8h[c][:], wv8h_d[c])

            # slot0 diag/row bias tiles + tri mask
            diag0 = pers.tile([128, W], F32, tag="diag0", bufs=1)
            rowt0 = pers.tile([128, W], F32, tag="rowt0", bufs=1)
            tri = pers.tile([128, 128], F32, tag="tri", bufs=1)

            def deferred_init():
                nc.sync.dma_start(ab_sb[:], ab_d[:])
                for h in range(HPC):
                    nc.sync.dma_start(
                        pk8_sb[h][:].rearrange("p r n -> p (r n)"),
                        pk8_d[:, 2 * PKOFF[h] * 128:
                              2 * (PKOFF[h] + NPK[h]) * 128])
                for h, dsrc, off in ((0, pv16_d, 0), (3, pv16_d, NPK[0]),
                                     (1, pvb_d, 0), (2, pvb_d, NPK[1])):
                    nc.sync.dma_start(
                        pv_sb[h].rearrange("p (c d) -> p c d", c=NPK[h]),
                        dsrc[off * 128:(off + NPK[h]) * 128]
                        .rearrange("(c p) d -> p c d", p=128),
                    )
                nc.sync.dma_start(diag0[:], dvb_d[0])
                nc.sync.dma_start(rowt0[:], dvb_d[1])
                nc.sync.dma_start(tri[:], mk_d[0])

            def qkv_groups(sc):
                """Return filler closures for supertile sc's QKV projection:
                one x-load group, 8 Q/K feature-chunk groups, 4 V groups."""
                x8t = [None] * 2
                xl8t = [None] * 2

                def xload():
                    for g in range(2):
                        t = x8pool.tile([128, 8, 2, W], F8, tag="x8",
                                        name=f"x8_{sc}_{g}")
                        # split slab DMAs so the first QK matmuls can start
                        # before the whole slab lands (finest for supertile 0
                        # where nothing else hides the latency)
                        nsp = 4 if sc == 0 else 2
                        for hh in range(nsp):
                            cw = 8 // nsp
                            c0 = g * 8 + hh * cw
                            nc.sync.dma_start(
                                t[:, hh * cw:(hh + 1) * cw],
                                x8_d[:, c0:c0 + cw, :, sc * W:(sc + 1) * W],
                            )
                        x8t[g] = t
                    for g in range(2):
                        tl = xl8pool.tile([128, 8, 2, W], F8, tag="xl8",
                                          name=f"xl8_{sc}_{g}")
                        nc.sync.dma_start(
                            tl[:],
                            xl8_d[:, g * 8:(g + 1) * 8, :, sc * W:(sc + 1) * W],
                        )
                        xl8t[g] = tl

                def qk_group(fc):
                    def emit():
                        psum = ps_qkv.tile([128, W], F32, tag="qkvps", name="qkvps")
                        for c in range(NPAIR):
                            nc.tensor.matmul(
                                psum[:],
                                w8[c][:, :, fc * 128:(fc + 1) * 128],
                                x8t[c // 8][:, c % 8, :, :],
                                start=(c == 0), stop=(c == NPAIR - 1),
                                perf_mode=mybir.MatmulPerfMode.DoubleRow,
                            )
                        if fc < 4:
                            qt = qpool.tile([128, 2, W], F8, tag=f"q{fc}",
                                            name=f"q{fc}_{sc}")
                            q_tiles[(fc, sc)] = qt
                            if sc < 2:
                                # first use of this pool buffer: make the
                                # r=1 plane finite (contents are irrelevant
                                # -- the stationary r=1 plane is zero)
                                nc.vector.memset(qt[:, 1, :], 0.0)
                            nc.vector.tensor_scalar_mul(
                                qt[:, 0, :], psum[:], QF)
                        else:
                            nc.vector.tensor_scalar_mul(
                                kT8[fc - 4][:, 0, sc * W:(sc + 1) * W],
                                psum[:], KF)
                    return emit

                def v_group(t4):
                    """V for key-chunk t4 of this supertile, directly in
                    [keys, 4*hd] layout: stationary = x pairs, moving = Wv."""
                    def emit():
                        psum = ps_qkv.tile([128, W], F32, tag="qkvps", name="qkvps")
                        nmm = 3 * NPAIR
                        i = 0
                        ksl = slice(t4 * 128, (t4 + 1) * 128)
                        for c in range(NPAIR):
                            xs = x8t[c // 8][:, c % 8, :, ksl]
                            xls = xl8t[c // 8][:, c % 8, :, ksl]
                            for (lhs, rhs) in (
                                (xs, wv8h[c]),
                                (xls, wv8h[c]),
                                (xs, wv8l[c]),
                            ):
                                nc.tensor.matmul(
                                    psum[:], lhs, rhs[:],
                                    start=(i == 0), stop=(i == nmm - 1),
                                    perf_mode=mybir.MatmulPerfMode.DoubleRow,
                                )
                                i += 1
                        t = sc * 4 + t4
                        # psum features are slot-major: [h0|h1|h2|h3]*128
                        nc.vector.tensor_scalar_mul(
                            vN16[t][:],
                            psum[:].rearrange("p (g d) -> p g d", g=4)
                            [:, 0::3, :],
                            SV_INV)
                        nc.vector.tensor_scalar_mul(
                            vNbf[t][:].rearrange("p r d -> p (r d)"),
                            psum[:, 128:384], SV_INV)
                    return emit

                return ([xload] + [qk_group(fc) for fc in range(8)]
                        + [v_group(t4) for t4 in range(4)])

            def oproj_groups(s):
                """o_proj partial rows for supertile s (32 closures; 3-term
                hi/lo fp8 DoubleRow over head pairs; batched weight loads
                prefetched one n-group ahead, 2-batched output stores on the
                Pool DMA queue)."""
                out = []
                opn = {}

                def wload(n):
                    th = opwpool.tile([128, 2, 2, W], F8, tag="opwh",
                                      name=f"opwh{n}_{s}")
                    nc.sync.dma_start(th[:], woh_d[n])
                    tl = opwpool.tile([128, 2, 2, W], F8, tag="opwl",
                                      name=f"opwl{n}_{s}")
                    nc.sync.dma_start(tl[:], wol_d[n])
                    opn[n] = (th, tl)

                tail = (s == 3)
                for n in range(8):
                    obt = [None]
                    for m4 in range(4):
                        def grp(n=n, m4=m4, obt=obt):
                            if n == 0 and m4 == 0 and 0 not in opn:
                                wload(0)
                            m = s * 4 + m4
                            if tail:
                                # attention is done: rotate over all psum
                                # pools so the obt drain never gates the PE
                                pspool = (ps_sh, ps_qkv, ps_s)[
                                    (n * 4 + m4) % 3]
                                psum = pspool.tile(
                                    [128, W], F32,
                                    tag={id(ps_sh): "sh", id(ps_qkv): "qkvps",
                                         id(ps_s): "sps"}[id(pspool)],
                                    name="shps")
                            else:
                                psum = ps_sh.tile([128, W], F32, tag="sh",
                                                  name="shps")
                            woh, wol = opn[n]
                            msl = slice(m4 * 128, (m4 + 1) * 128)
                            i = 0
                            for hp in range(2):
                                ath, atl = at8_tiles[(hp, s)]
                                for (lhs, rhs) in (
                                    (ath, woh), (atl, woh), (ath, wol),
                                ):
                                    nc.tensor.matmul(
                                        psum[:],
                                        lhs[:, :, msl],
                                        rhs[:, hp, :, :],
                                        start=(i == 0), stop=(i == 5),
                                        perf_mode=mybir.MatmulPerfMode.DoubleRow,
                                    )
                                    i += 1
                            if m4 == 0 and n + 1 < 8:
                                wload(n + 1)

                            if m4 % 2 == 0:
                                obt[0] = obpool.tile([128, 2, W], F16,
                                                     tag="ob", name="ob")
                            # alternate engines so the psum drain pipelines
                            if m4 % 2 == 0:
                                nc.vector.tensor_scalar_mul(
                                    obt[0][:, 0, :], psum[:], SO_INV)
                            else:
                                nc.scalar.activation(
                                    obt[0][:, 1, :], psum[:],
                                    mybir.ActivationFunctionType.Copy,
                                    scale=SO_INV)
                            if m4 % 2 == 1:
                                # tail: two store queues so the final drain
                                # pipelines (ACT is a hwdge engine too)
                                eng = (nc.scalar if tail and n % 2 == 1
                                       else nc.gpsimd)
                                eng.dma_start(
                                    out_d[(m - 1) * 128:(m + 1) * 128,
                                          n * W:(n + 1) * W]
                                    .rearrange("(two p) f -> p two f", p=128),
                                    obt[0][:],
                                )
                        out.append(grp)
                return out, (lambda: wload(0) if 0 not in opn else None)

            def attention_all(s, fillers):
                """All heads for supertile s, software-pipelined (scores run
                DEPTH tiles ahead of PV) with filler groups interleaved to
                keep the PE busy during the add->exp latency chain."""
                nvis = 12 + 4 * s
                # zero-DVE slots first so the phase starts with pure
                # PE-stream tiles while the DVE/ACT queues drain
                tiles = [(h, j) for h in (3, 2, 1, 0)
                         for j in range(JMIN[h][s], nvis)]
                DEPTH = 2 if s == 1 else 6
                ntiles = len(tiles)
                nfill = len(fillers)
                filled = 0
                state = {}   # h -> (o_ps, acc)
                pend = []    # [(h, j, p, off, nv)]
                scnt = 0

                def emit_scores(h, j):
                    nonlocal scnt
                    m = j - (8 + 4 * s)
                    off = 0 if m < 0 else 128 * m
                    nv = W - off
                    if s >= 2 and scnt % 2 == 1:
                        sp = ps_qkv.tile([128, W], F32, tag="qkvps", name="qkvps")
                    elif s == 0 and scnt % 2 == 1:
                        sp = ps_sh.tile([128, W], F32, tag="sh", name="shps")
                    else:
                        sp = ps_s.tile([128, W], F32, tag="sps", name="sps")
                    scnt += 1
                    if j < 8:
                        jj = j - PK0[h]
                        kt = pk8_sb[h][:, :, jj * 128:(jj + 1) * 128]
                    else:
                        kt = kT8[h][:, :, (j - 8) * 128:(j - 7) * 128]
                    nc.tensor.matmul(
                        sp[:, :nv], kt,
                        q_tiles[(h, s)][:, :, off:],
                        start=True, stop=True,
                        perf_mode=mybir.MatmulPerfMode.DoubleRow,
                    )
                    p = ppool.tile([128, W], BF16 if SLOT_BF[h] else F16,
                                   tag="p", name="p")
                    col = (h * NST + s) * NKT + j
                    if h == 0:
                        # steep slope: exact per-query row bias (+tri on diag)
                        s1 = s1pool.tile([128, W], F32, tag="s1", bufs=2,
                                             name="s1")
                        rt = diag0 if m >= 0 else rowt0
                        nc.vector.tensor_add(s1[:, :nv], sp[:, :nv],
                                             rt[:, :nv])
                        src = s1
                    else:
                        if m >= 0:
                            # causal mask on the diagonal 128 block, in-place
                            nc.vector.tensor_add(sp[:, :128], sp[:, :128],
                                                 tri[:])
                        src = sp
                    nc.scalar.activation(
                        p[:, :nv], src[:, :nv],
                        mybir.ActivationFunctionType.Exp,
                        bias=ab_sb[:, col:col + 1],
                        scale=AEXP,
                    )
                    pend.append((h, j, p, off, nv))

                def emit_pv():
                    h, j, p, off, nv = pend.pop(0)
                    j0 = JMIN[h][s]
                    if j == j0:
                        o_ps = ps_o.tile([128, W], F32, tag="ops",
                                         name=f"ops{h}")
                        acc = daccpool.tile([128, W],
                                            BF16 if SLOT_BF[h] else F16,
                                            tag="dacc", name=f"dacc{h}_{s}")
                        state[h] = (o_ps, acc)
                    o_ps, acc = state[h]
                    if j < 8:
                        jj = j - PK0[h]
                        vt = pv_sb[h][:, jj * 128:(jj + 1) * 128]
                    elif SLOT_BF[h]:
                        vt = vNbf[j - 8][:, VNB_IDX[h], :]
                    else:
                        vt = vN16[j - 8][:, VN_IDX[h], :]
                    nc.tensor.matmul(
                        o_ps[:, off:], vt, p[:, :nv],
                        start=(j == j0), stop=(j == nvis - 1),
                    )
                    # denominator accumulation on DVE (fp16, 2x mode)
                    if j == j0:
                        if off:
                            nc.vector.memset(acc[:, :off], 0.0)
                        nc.vector.tensor_copy(acc[:, off:], p[:, :nv])
                    else:
                        nc.vector.tensor_add(
                            acc[:, off:], acc[:, off:], p[:, :nv])
                    if j == nvis - 1:
                        # denominator + normalize (d reuses a scores psum slot)
                        # ones16 = 1/SAT so bb = SAT/den and at16 = at*SAT.
                        d_ps = ps_s.tile([128, W], F32, tag="sps", name="dps")
                        nc.tensor.matmul(
                            d_ps[0:1, :],
                            (onesbf if SLOT_BF[h] else ones16)[:], acc[:],
                            start=True, stop=True,
                        )
                        denr = smallpool.tile([1, W], F32, tag="denr",
                                              name="denr")
                        nc.vector.reciprocal(denr[:], d_ps[0:1, :])
                        bb = s1pool.tile([128, W], F32, tag="bb", bufs=2,
                                         name="bb")
                        nc.gpsimd.partition_broadcast(bb[:], denr[:])
                        at16 = atpool.tile([128, W], F16, tag="at16",
                                           name=f"at16_{h}_{s}")
                        nc.vector.tensor_mul(at16[:], o_ps[:], bb[:])
                        # hi/lo fp8 split for the o_proj DoubleRow matmuls
                        hp, r = h // 2, h % 2
                        if (hp, s) not in at8_tiles:
                            ath = atpool.tile([128, 2, W], F8, tag=f"a8h{hp}",
                                              name=f"a8h{hp}_{s}")
                            atl = atpool.tile([128, 2, W], F8, tag=f"a8l{hp}",
                                              name=f"a8l{hp}_{s}")
                            at8_tiles[(hp, s)] = (ath, atl)
                        ath, atl = at8_tiles[(hp, s)]
                        nc.scalar.activation(
                            ath[:, r, :], at16[:],
                            mybir.ActivationFunctionType.Copy)
                        nc.vector.tensor_sub(atl[:, r, :], at16[:], ath[:, r, :])

                for idx, (h, j) in enumerate(tiles):
                    emit_scores(h, j)
                    # interleave filler work proportionally
                    want = (idx + 1) * nfill // ntiles
                    while filled < want:
                        fillers[filled]()
                        filled += 1
                    if len(pend) >= DEPTH:
                        emit_pv()
                while pend:
                    emit_pv()
                while filled < nfill:
                    fillers[filled]()
                    filled += 1

            # ---- pipelined emission order ----
            for g in qkv_groups(0):
                g()
            # zero the r=1 plane of kT8 once (DVE idles here); scores
            # matmuls contract over [128, 2] with an all-zero r=1 plane
            # (moving-side garbage multiplies the zeros, so it's inert).
            for h in range(HPC):
                nc.gpsimd.memset(kT8[h][:, 1, :], 0.0)
            for g in qkv_groups(1):
                g()
            # attention tables ride the SP queue behind both x-slab sets
            # (needed only when attention_all(0) starts)
            deferred_init()
            # x slabs prefetched one phase ahead of their filler groups
            g2 = qkv_groups(2)
            g2[0]()
            attention_all(0, g2[1:])
            g3 = qkv_groups(3)
            g3[0]()
            op0, _ = oproj_groups(0)
            attention_all(1, g3[1:] + op0)
            op1, _ = oproj_groups(1)
            attention_all(2, op1)
            op2, _ = oproj_groups(2)
            op3, op3_pre = oproj_groups(3)
            # prefetch the tail's first o_proj weight tiles during the last
            # attention phase so the tail doesn't start with a stall
            attention_all(3, op2 + [op3_pre])
            for g in op3:
                g()

    return nc


_CACHE = {}


def _build():
    if "nc" not in _CACHE:
        nc = bacc.Bacc(
            trn_type="TRN2", target_bir_lowering=False, debug=False,
            num_devices=NCORES,
        )
        _emit(nc)
        nc.compile()
        _CACHE["nc"] = nc
    return _CACHE["nc"]


def _pair8(a):
    """[D, F] -> fp8 pair layout [NPAIR, 128, 2, F]."""
    Dd, F = a.shape
    return np.ascontiguousarray(
        a.reshape(NPAIR, 2, 128, F).transpose(0, 2, 1, 3)
    ).astype(NPF8)


def _pair8_pm(a):
    """[D, F] -> fp8 partition-major pair layout [128, NPAIR, 2, F]."""
    Dd, F = a.shape
    return np.ascontiguousarray(
        a.reshape(NPAIR, 2, 128, F).transpose(2, 0, 1, 3)
    ).astype(NPF8)


def _host_prep(hidden_states, past_key, past_value, W_pack_w, o_proj_w):
    x = np.asarray(hidden_states, np.float32).reshape(S, D)
    pk = np.asarray(past_key, np.float32).reshape(H, HIST, HD)
    pv = np.asarray(past_value, np.float32).reshape(H, HIST, HD)
    Wp = np.asarray(W_pack_w, np.float32)
    Wo = np.asarray(o_proj_w, np.float32)
    slopes = _alibi_slopes(H)

    xT = np.ascontiguousarray(x.T)
    xs = xT * SX
    xh = xs.astype(NPF8).astype(np.float32)
    x8 = _pair8_pm(xh)                              # hi (exactly representable)
    xl8 = _pair8_pm(xs - xh)                        # lo residual

    scale = np.float32(1.0 / np.sqrt(HD))
    kk = np.arange(128, dtype=np.float32)
    ii = np.arange(W, dtype=np.float32)

    in_maps = []
    for c in range(NCORES):
        heads = [8 * i + c for i in range(HPC)]
        rsel = np.concatenate(
            [np.arange(hh * HD, (hh + 1) * HD) for hh in heads])
        Wq = Wp[rsel] * scale
        Wk = Wp[D + rsel]
        Wv = Wp[2 * D + rsel]
        Wqk = np.concatenate([Wq * SWQ, Wk * SWK], 0).T  # [D, 1024]
        w8 = _pair8(Wqk)
        wvs = Wv.T * SWV                                 # [D, FPC]
        wvh = wvs.astype(NPF8).astype(np.float32)
        wv8h = _pair8(wvh)
        wv8l = _pair8(wvs - wvh)
        # flat truncated past-K in fp8 [128, 2, n*128] per slot, r=1 zeroed
        pk8_segs = []
        for h in range(HPC):
            seg = np.zeros((128, 2, NPK[h] * 128), np.float32)
            seg[:, 0, :] = pk[heads[h]][PK0[h] * 128:, :].T * SKC
            pk8_segs.append(seg.reshape(128, -1))
        pk8 = np.ascontiguousarray(
            np.concatenate(pk8_segs, axis=1)).astype(NPF8)
        pv16c = np.ascontiguousarray(np.concatenate(
            [pv[heads[h]][PK0[h] * 128:, :] for h in (0, 3)], axis=0
        )).astype(np.float16)
        pvbc = np.ascontiguousarray(np.concatenate(
            [pv[heads[h]][PK0[h] * 128:, :] for h in (1, 2)], axis=0
        )).astype(NPBF)
        # o_proj weights, hi/lo fp8, head-pair DoubleRow layout
        # [n, p, hp, r, W] (pre-sliced by 512-wide output group)
        wos = np.zeros((128, 2, 2, D), np.float32)
        for hh in range(HPC):
            wos[:, hh // 2, hh % 2, :] = (
                Wo[:, heads[hh] * HD:(heads[hh] + 1) * HD].T * SWO)
        wos = np.ascontiguousarray(
            wos.reshape(128, 2, 2, 8, W).transpose(3, 0, 1, 2, 4))
        woh8 = wos.astype(NPF8)
        wol8 = (wos - woh8.astype(np.float32)).astype(NPF8)
        sl = slopes[heads]

        ab = np.zeros((HPC, NST, NKT, 128), np.float32)
        for h in range(HPC):
            for s in range(NST):
                for j in range(NKT):
                    ab[h, s, j] = sl[h] * (128 * j + kk - HIST - W * s)
                    m = j - (8 + 4 * s)
                    if h == 0 and m >= 0:
                        # diag0 tile carries -sl*c; shift the per-key bias
                        # so the pair reproduces sl*(k_pos - q_pos)
                        ab[h, s, j] -= sl[h] * 128 * m
            if CQ[h] is not None:
                # per-query recentering constant (softmax-invariant)
                ab[h] -= sl[h] * CQ[h]
        ab_sb = np.ascontiguousarray(
            ab.reshape(HPC * NST * NKT, 128).T
        )

        mkpat = np.where(ii[None, :128] >= kk[:, None], 0.0,
                         NEG).astype(np.float32)
        dvb = np.zeros((2, 128, W), np.float32)
        dvb[0, :, :128] = mkpat
        dvb[0] += (np.float32(RS) * -sl[0] * ii)[None, :]
        dvb[1] = (np.float32(RS) * -sl[0] * ii)[None, :]
        mk = mkpat[None]
        in_maps.append({
            "x8": x8, "xl8": xl8, "w8": w8, "wv8h": wv8h, "wv8l": wv8l,
            "pk8": pk8, "pv16": pv16c, "pvb": pvbc,
            "woh8": woh8, "wol8": wol8,
            "abias": ab_sb, "dvb": dvb, "mk": mk,
        })
    return in_maps


def kernel(hidden_states, past_key, past_value, W_pack_w, o_proj_w):
    nc = _build()
    in_maps = _host_prep(hidden_states, past_key, past_value, W_pack_w, o_proj_w)
    res = run_bass_kernel_spmd(nc, in_maps, list(range(NCORES)))
    out = np.zeros((S, D), np.float64)
    for c in range(NCORES):
        out += res.results[c]["outp"].astype(np.float64)
    return out.astype(np.float32).reshape(B, S, D)



# revision 68
# speedup vs baseline: 1.0242x; 1.0242x over previous
"""Trainium2 Bass kernel for nn_BaichuanAttention (ALiBi attention + KV cache).

Head-parallel across 8 NeuronCores (4 heads/core). Per core:
  - Q/K projection: fp8e4 DoubleRow matmuls (256-deep contraction at
    0.5 cycles/row); weights resident in SBUF. Scores insensitive to
    fp8 rounding (ALiBi dominates the softmax).
  - V projection: 3-term hi/lo fp8 DoubleRow (x = xh + xl, Wv = wh + wl,
    v ~= wh@xh + wh@xl + wl@xh) giving ~fp16 accuracy at 0.75x the fp16
    matmul cost.
  - attention with transposed scores [keys, queries]:
      * scores via fp8 DoubleRow matmuls (q/k requantized to fp8; the
        stationary r=1 plane is zero so the 256-deep DR contraction
        reduces to the real 128-deep hd one at 0.5 cycles/row)
      * ALiBi per-key term folded into the ACT exp() per-partition bias
      * ALiBi per-query term + causal mask applied via DVE adds
      * softmax denominator via ones-matmul on the PE (ones=1/SAT so the
        normalized attention output comes out pre-scaled by SAT)
  - PV accumulation (fp16), normalize via outer-product broadcast
  - o_proj: 3-term hi/lo fp8 DoubleRow over head pairs
    (ath@woh + atl@woh + ath@wol), psum descaled by 1/(SAT*SWO)
Host: shard/transpose/cast/scale inputs, sum fp16 o_proj partials.
"""
import os
import sys

import numpy as np

for _p in ("/opt/trn_rl_repo",):
    if os.path.isdir(_p) and _p not in sys.path:
        sys.path.insert(0, _p)

import ml_dtypes
import concourse.bass as bass
import concourse.mybir as mybir
import concourse.tile as tile
from concourse import bacc
from concourse.bass_utils import run_bass_kernel_spmd
from concourse.masks import make_identity

F32 = mybir.dt.float32
F16 = mybir.dt.float16
BF16 = mybir.dt.bfloat16
F8 = mybir.dt.float8e4
NPF8 = ml_dtypes.float8_e4m3
NPBF = ml_dtypes.bfloat16

B, S, D, H, HD, HIST = 1, 2048, 4096, 32, 128, 1024
T = HIST + S
NCORES = 8
HPC = H // NCORES          # heads per core
FPC = HPC * HD             # 512 features per core per section
NST = S // 512             # 4 query supertiles
W = 512                    # supertile width
NKT = T // 128             # 24 key tiles
NKC = D // 128             # 32 contraction chunks for QKV
NPAIR = NKC // 2           # 16 DoubleRow contraction pairs
NEG = -1.0e30

SX = 128.0                 # x fp8 scale
SWQ = 2048.0               # Wq (incl 1/sqrt(hd)) fp8 scale
SWK = 128.0                # Wk fp8 scale
SWV = 128.0                # Wv fp8 scale
SV_INV = 1.0 / (SX * SWV)  # V psum descale
# q/k requantized to fp8 for DoubleRow scores (zero-padded r=1 plane);
# uniform scale for past and new keys.
SQC = 8192.0               # q fp8 scale
SKC = 32.0                 # k fp8 scale (past |k|~5.5 sigma * 32 < 240)
QF = SQC / (SX * SWQ)      # q psum -> fp8 rescale
KF = SKC / (SX * SWK)      # k psum -> fp8 rescale
RS = SQC * SKC             # scores psum scale (all tiles)
AEXP = 1.0 / RS            # ACT exp input descale
SWO = 128.0                # Wo fp8 scale
SAT = 32.0                 # attn-out fp8 scale (folded into 1/den via ones)
SO_INV = 1.0 / (SAT * SWO)  # o_proj psum descale


def _alibi_slopes(n_heads: int) -> np.ndarray:
    def pow2_slopes(m):
        start = 2.0 ** (-(2.0 ** -(np.log2(m) - 3)))
        return start * (start ** np.arange(m))
    if np.log2(n_heads).is_integer():
        return pow2_slopes(n_heads).astype(np.float32)
    m = 2 ** int(np.floor(np.log2(n_heads)))
    base = pow2_slopes(m)
    extra = pow2_slopes(2 * m)[0::2][: n_heads - m]
    return np.concatenate([base, extra]).astype(np.float32)


# --- ALiBi window truncation ---------------------------------------------
# Keys further than MARGIN/slope behind a query contribute exp(-MARGIN)
# relative weight — drop their tiles. Heads are distributed so core c gets
# heads {c, 8+c, 16+c, 24+c} (slot i = head 8i+c); each slot's window is
# sized for the *smallest* slope in its group, so every core runs an
# identical instruction stream.
MARGIN = 11.0
_SLOPES_ALL = _alibi_slopes(H)
JMIN = []
for _i in range(HPC):
    _win = MARGIN / float(_SLOPES_ALL[8 * _i + 7])
    JMIN.append([
        max(0, min(12 + 4 * _s, int((HIST + W * _s - _win) // 128)))
        for _s in range(NST)
    ])
# past-key/value tiles actually reachable per slot (j in [PK0[h], 8))
PK0 = [min(JMIN[_i][0], 8) for _i in range(HPC)]
NPK = [8 - PK0[_i] for _i in range(HPC)]
PKOFF = [sum(NPK[:_i]) for _i in range(HPC)]   # segment offsets, in tiles
NPKT = sum(NPK)

# --- per-slot softmax numerics -------------------------------------------
# Softmax is invariant to any per-query constant added to all visible
# scores; the per-query ALiBi term -sl*q_pos is only needed to avoid
# overflow.  Per slot we use the cheapest scheme the slope allows:
#   slot 0 (steep): exact -sl*i row bias via DVE add, p fp16
#   slots 1,2:      constant recenter C=sl*256, p bf16 (range to e^54)
#   slot 3 (flat):  constant recenter C=sl*128, p fp16 (max e^5)
# so slots 1-3 need NO DVE op on non-diagonal tiles (ACT reads psum).
CQ = [None, 256.0, 256.0, 128.0]
SLOT_BF = [False, True, True, False]
# fp16 slots 0,3 share vN16 (r: 0->slot0, 1->slot3); bf16 slots share vNbf
VN_IDX = {0: 0, 3: 1}
VNB_IDX = {1: 0, 2: 1}
NPK16 = NPK[0] + NPK[3]
NPKBF = NPK[1] + NPK[2]


def _emit(nc):
    """Emit the whole per-core program under a TileContext."""
    x8_d = nc.dram_tensor("x8", [128, NPAIR, 2, S], F8, kind="ExternalInput").ap()
    xl8_d = nc.dram_tensor("xl8", [128, NPAIR, 2, S], F8, kind="ExternalInput").ap()
    w8_d = nc.dram_tensor("w8", [NPAIR, 128, 2, 1024], F8, kind="ExternalInput").ap()
    wv8h_d = nc.dram_tensor("wv8h", [NPAIR, 128, 2, FPC], F8, kind="ExternalInput").ap()
    wv8l_d = nc.dram_tensor("wv8l", [NPAIR, 128, 2, FPC], F8, kind="ExternalInput").ap()
    pk8_d = nc.dram_tensor("pk8", [128, 2 * NPKT * 128], F8, kind="ExternalInput").ap()
    pv16_d = nc.dram_tensor("pv16", [NPK16 * 128, HD], F16, kind="ExternalInput").ap()
    pvb_d = nc.dram_tensor("pvb", [NPKBF * 128, HD], BF16, kind="ExternalInput").ap()
    woh_d = nc.dram_tensor("woh8", [8, 128, 2, 2, W], F8, kind="ExternalInput").ap()
    wol_d = nc.dram_tensor("wol8", [8, 128, 2, 2, W], F8, kind="ExternalInput").ap()
    ab_d = nc.dram_tensor("abias", [128, HPC * NST * NKT], F32, kind="ExternalInput").ap()
    # dvb[0] = slot0 diag add (tri + -sl*c*RS), dvb[1] = slot0 row bias
    dvb_d = nc.dram_tensor("dvb", [2, 128, W], F32, kind="ExternalInput").ap()
    mk_d = nc.dram_tensor("mk", [1, 128, 128], F32, kind="ExternalInput").ap()
    out_d = nc.dram_tensor("outp", [S, D], F16, kind="ExternalOutput").ap()

    with tile.TileContext(nc) as tc:
        with (
            tc.tile_pool(name="persist", bufs=1) as pers,
            tc.tile_pool(name="x8slab", bufs=2) as x8pool,
            tc.tile_pool(name="xl8slab", bufs=2) as xl8pool,
            tc.tile_pool(name="qp", bufs=2) as qpool,
            tc.tile_pool(name="opwp", bufs=3) as opwpool,
            tc.tile_pool(name="s1", bufs=3) as s1pool,
            tc.tile_pool(name="pp", bufs=7) as ppool,
            tc.tile_pool(name="dac", bufs=2) as daccpool,
            tc.tile_pool(name="small", bufs=1) as smallpool,
            tc.tile_pool(name="ob", bufs=3) as obpool,
            tc.tile_pool(name="at", bufs=2) as atpool,
            tc.tile_pool(name="ps_qkv", bufs=2, space="PSUM") as ps_qkv,
            tc.tile_pool(name="ps_s", bufs=2, space="PSUM") as ps_s,
            tc.tile_pool(name="ps_o", bufs=2, space="PSUM") as ps_o,
            tc.tile_pool(name="ps_sh", bufs=2, space="PSUM") as ps_sh,
        ):
            # ---- persistent SBUF tensors ----
            # k in fp8 for DoubleRow scores: r=1 plane kept zero so the
            # 256-deep DR contraction reduces to the real 128-deep one.
            kT8 = [pers.tile([128, 2, S], F8, tag=f"kT{h}", bufs=1, name=f"kT{h}") for h in range(HPC)]
            # new-v per 128-key chunk: fp16 slots {0,3} and bf16 slots {1,2}
            vN16 = [pers.tile([128, 2, HD], F16, tag=f"vN{t}", bufs=1, name=f"vN{t}")
                    for t in range(S // 128)]
            vNbf = [pers.tile([128, 2, HD], BF16, tag=f"vB{t}", bufs=1, name=f"vB{t}")
                    for t in range(S // 128)]
            pk8_sb = [pers.tile([128, 2, NPK[h] * 128], F8, tag=f"pk{h}", bufs=1, name=f"pk{h}") for h in range(HPC)]
            pv_sb = [pers.tile([128, NPK[h] * 128],
                               BF16 if SLOT_BF[h] else F16,
                               tag=f"pvs{h}", bufs=1, name=f"pvs{h}")
                     for h in range(HPC)]
            at8_tiles = {}
            q_tiles = {}
            ab_sb = pers.tile([128, HPC * NST * NKT], F32, tag="abias", bufs=1)
            ones16 = pers.tile([128, 1], F16, tag="ones16", bufs=1)
            onesbf = pers.tile([128, 1], BF16, tag="onesbf", bufs=1)
            # resident fp8 weights
            w8 = [pers.tile([128, 2, 1024], F8, tag=f"w8_{c}", bufs=1, name=f"w8_{c}")
                  for c in range(NPAIR)]
            wv8h = [pers.tile([128, 2, FPC], F8, tag=f"wvh{c}", bufs=1, name=f"wvh{c}")
                    for c in range(NPAIR)]
            wv8l = [pers.tile([128, 2, FPC], F8, tag=f"wvl{c}", bufs=1, name=f"wvl{c}")
                    for c in range(NPAIR)]

            nc.any.memset(ones16[:], 1.0 / SAT)
            nc.any.memset(onesbf[:], 1.0 / SAT)
            # w8 is the startup-critical load: split it over the Pool and
            # ACT queues.  wv8l rides Pool behind it; everything else is
            # deferred onto the SP queue after supertile 0's x slabs (see
            # deferred_init below) so the ACT/DVE queues start empty.
            for c in range(NPAIR):
                # alternate queues so w8 arrival keeps pace with consumption
                (nc.gpsimd if c % 2 == 0 else nc.scalar).dma_start(
                    w8[c][:], w8_d[c])
            for c in range(NPAIR):
                nc.gpsimd.dma_start(wv8l[c][:], wv8l_d[c])
                nc.scalar.dma_start(wv8h[c][:], wv8h_d[c])

            # slot0 diag/row bias tiles + tri mask
            diag0 = pers.tile([128, W], F32, tag="diag0", bufs=1)
            rowt0 = pers.tile([128, W], F32, tag="rowt0", bufs=1)
            tri = pers.tile([128, 128], F32, tag="tri", bufs=1)

            def deferred_init():
                nc.sync.dma_start(ab_sb[:], ab_d[:])
                for h in range(HPC):
                    nc.sync.dma_start(
                        pk8_sb[h][:].rearrange("p r n -> p (r n)"),
                        pk8_d[:, 2 * PKOFF[h] * 128:
                              2 * (PKOFF[h] + NPK[h]) * 128])
                for h, dsrc, off in ((0, pv16_d, 0), (3, pv16_d, NPK[0]),
                                     (1, pvb_d, 0), (2, pvb_d, NPK[1])):
                    nc.sync.dma_start(
                        pv_sb[h].rearrange("p (c d) -> p c d", c=NPK[h]),
                        dsrc[off * 128:(off + NPK[h]) * 128]
                        .rearrange("(c p) d -> p c d", p=128),
                    )
                nc.sync.dma_start(diag0[:], dvb_d[0])
                nc.sync.dma_start(rowt0[:], dvb_d[1])
                nc.sync.dma_start(tri[:], mk_d[0])

            def qkv_groups(sc):
                """Return filler closures for supertile sc's QKV projection:
                one x-load group, 8 Q/K feature-chunk groups, 4 V groups."""
                x8t = [None] * 2
                xl8t = [None] * 2

                def xload():
                    for g in range(2):
                        t = x8pool.tile([128, 8, 2, W], F8, tag="x8",
                                        name=f"x8_{sc}_{g}")
                        # split slab DMAs so the first QK matmuls can start
                        # before the whole slab lands (finest for supertile 0
                        # where nothing else hides the latency)
                        nsp = 4 if sc == 0 else 2
                        for hh in range(nsp):
                            cw = 8 // nsp
                            c0 = g * 8 + hh * cw
                            nc.sync.dma_start(
                                t[:, hh * cw:(hh + 1) * cw],
                                x8_d[:, c0:c0 + cw, :, sc * W:(sc + 1) * W],
                            )
                        x8t[g] = t
                    for g in range(2):
                        tl = xl8pool.tile([128, 8, 2, W], F8, tag="xl8",
                                          name=f"xl8_{sc}_{g}")
                        nc.sync.dma_start(
                            tl[:],
                            xl8_d[:, g * 8:(g + 1) * 8, :, sc * W:(sc + 1) * W],
                        )
                        xl8t[g] = tl

                def qk_group(fc):
                    def emit():
                        psum = ps_qkv.tile([128, W], F32, tag="qkvps", name="qkvps")
                        for c in range(NPAIR):
                            nc.tensor.matmul(
                                psum[:],
                                w8[c][:, :, fc * 128:(fc + 1) * 128],
                                x8t[c // 8][:, c % 8, :, :],
                                start=(c == 0), stop=(c == NPAIR - 1),
                                perf_mode=mybir.MatmulPerfMode.DoubleRow,
                            )
                        if fc < 4:
                            qt = qpool.tile([128, 2, W], F8, tag=f"q{fc}",
                                            name=f"q{fc}_{sc}")
                            q_tiles[(fc, sc)] = qt
                            if sc < 2:
                                # first use of this pool buffer: make the
                                # r=1 plane finite (contents are irrelevant
                                # -- the stationary r=1 plane is zero)
                                nc.vector.memset(qt[:, 1, :], 0.0)
                            nc.vector.tensor_scalar_mul(
                                qt[:, 0, :], psum[:], QF)
                        else:
                            nc.vector.tensor_scalar_mul(
                                kT8[fc - 4][:, 0, sc * W:(sc + 1) * W],
                                psum[:], KF)
                    return emit

                def v_group(t4):
                    """V for key-chunk t4 of this supertile, directly in
                    [keys, 4*hd] layout: stationary = x pairs, moving = Wv."""
                    def emit():
                        psum = ps_qkv.tile([128, W], F32, tag="qkvps", name="qkvps")
                        nmm = 3 * NPAIR
                        i = 0
                        ksl = slice(t4 * 128, (t4 + 1) * 128)
                        for c in range(NPAIR):
                            xs = x8t[c // 8][:, c % 8, :, ksl]
                            xls = xl8t[c // 8][:, c % 8, :, ksl]
                            for (lhs, rhs) in (
                                (xs, wv8h[c]),
                                (xls, wv8h[c]),
                                (xs, wv8l[c]),
                            ):
                                nc.tensor.matmul(
                                    psum[:], lhs, rhs[:],
                                    start=(i == 0), stop=(i == nmm - 1),
                                    perf_mode=mybir.MatmulPerfMode.DoubleRow,
                                )
                                i += 1
                        t = sc * 4 + t4
                        # psum features are slot-major: [h0|h1|h2|h3]*128
                        nc.vector.tensor_scalar_mul(
                            vN16[t][:],
                            psum[:].rearrange("p (g d) -> p g d", g=4)
                            [:, 0::3, :],
                            SV_INV)
                        nc.vector.tensor_scalar_mul(
                            vNbf[t][:].rearrange("p r d -> p (r d)"),
                            psum[:, 128:384], SV_INV)
                    return emit

                return ([xload] + [qk_group(fc) for fc in range(8)]
                        + [v_group(t4) for t4 in range(4)])

            def oproj_groups(s):
                """o_proj partial rows for supertile s (32 closures; 3-term
                hi/lo fp8 DoubleRow over head pairs; batched weight loads
                prefetched one n-group ahead, 2-batched output stores on the
                Pool DMA queue)."""
                out = []
                opn = {}

                def wload(n):
                    th = opwpool.tile([128, 2, 2, W], F8, tag="opwh",
                                      name=f"opwh{n}_{s}")
                    nc.sync.dma_start(th[:], woh_d[n])
                    tl = opwpool.tile([128, 2, 2, W], F8, tag="opwl",
                                      name=f"opwl{n}_{s}")
                    nc.sync.dma_start(tl[:], wol_d[n])
                    opn[n] = (th, tl)

                tail = (s == 3)
                for n in range(8):
                    obt = [None]
                    for m4 in range(4):
                        def grp(n=n, m4=m4, obt=obt):
                            if n == 0 and m4 == 0 and 0 not in opn:
                                wload(0)
                            m = s * 4 + m4
                            if tail:
                                # attention is done: rotate over all psum
                                # pools so the obt drain never gates the PE
                                pspool = (ps_sh, ps_qkv, ps_s)[
                                    (n * 4 + m4) % 3]
                                psum = pspool.tile(
                                    [128, W], F32,
                                    tag={id(ps_sh): "sh", id(ps_qkv): "qkvps",
                                         id(ps_s): "sps"}[id(pspool)],
                                    name="shps")
                            else:
                                psum = ps_sh.tile([128, W], F32, tag="sh",
                                                  name="shps")
                            woh, wol = opn[n]
                            msl = slice(m4 * 128, (m4 + 1) * 128)
                            i = 0
                            for hp in range(2):
                                ath, atl = at8_tiles[(hp, s)]
                                for (lhs, rhs) in (
                                    (ath, woh), (atl, woh), (ath, wol),
                                ):
                                    nc.tensor.matmul(
                                        psum[:],
                                        lhs[:, :, msl],
                                        rhs[:, hp, :, :],
                                        start=(i == 0), stop=(i == 5),
                                        perf_mode=mybir.MatmulPerfMode.DoubleRow,
                                    )
                                    i += 1
                            if m4 == 0 and n + 1 < 8:
                                wload(n + 1)

                            if m4 % 2 == 0:
                                obt[0] = obpool.tile([128, 2, W], F16,
                                                     tag="ob", name="ob")
                            # alternate engines so the psum drain pipelines
                            if m4 % 2 == 0:
                                nc.vector.tensor_scalar_mul(
                                    obt[0][:, 0, :], psum[:], SO_INV)
                            else:
                                nc.scalar.activation(
                                    obt[0][:, 1, :], psum[:],
                                    mybir.ActivationFunctionType.Copy,
                                    scale=SO_INV)
                            if m4 % 2 == 1:
                                # tail: two store queues so the final drain
                                # pipelines (ACT is a hwdge engine too)
                                eng = (nc.scalar if tail and n % 2 == 1
                                       else nc.gpsimd)
                                eng.dma_start(
                                    out_d[(m - 1) * 128:(m + 1) * 128,
                                          n * W:(n + 1) * W]
                                    .rearrange("(two p) f -> p two f", p=128),
                                    obt[0][:],
                                )
                        out.append(grp)
                return out, (lambda: wload(0) if 0 not in opn else None)

            def attention_all(s, fillers):
                """All heads for supertile s, software-pipelined (scores run
                DEPTH tiles ahead of PV) with filler groups interleaved to
                keep the PE busy during the add->exp latency chain."""
                nvis = 12 + 4 * s
                tiles = [(h, j) for h in range(HPC)
                         for j in range(JMIN[h][s], nvis)]
                DEPTH = 2 if s == 1 else 6
                ntiles = len(tiles)
                nfill = len(fillers)
                filled = 0
                state = {}   # h -> (o_ps, acc)
                pend = []    # [(h, j, p, off, nv)]
                scnt = 0

                def emit_scores(h, j):
                    nonlocal scnt
                    m = j - (8 + 4 * s)
                    off = 0 if m < 0 else 128 * m
                    nv = W - off
                    if s >= 2 and scnt % 2 == 1:
                        sp = ps_qkv.tile([128, W], F32, tag="qkvps", name="qkvps")
                    elif s == 0 and scnt % 2 == 1:
                        sp = ps_sh.tile([128, W], F32, tag="sh", name="shps")
                    else:
                        sp = ps_s.tile([128, W], F32, tag="sps", name="sps")
                    scnt += 1
                    if j < 8:
                        jj = j - PK0[h]
                        kt = pk8_sb[h][:, :, jj * 128:(jj + 1) * 128]
                    else:
                        kt = kT8[h][:, :, (j - 8) * 128:(j - 7) * 128]
                    nc.tensor.matmul(
                        sp[:, :nv], kt,
                        q_tiles[(h, s)][:, :, off:],
                        start=True, stop=True,
                        perf_mode=mybir.MatmulPerfMode.DoubleRow,
                    )
                    p = ppool.tile([128, W], BF16 if SLOT_BF[h] else F16,
                                   tag="p", name="p")
                    col = (h * NST + s) * NKT + j
                    if h == 0:
                        # steep slope: exact per-query row bias (+tri on diag)
                        s1 = s1pool.tile([128, W], F32, tag="s1", bufs=2,
                                             name="s1")
                        rt = diag0 if m >= 0 else rowt0
                        nc.vector.tensor_add(s1[:, :nv], sp[:, :nv],
                                             rt[:, :nv])
                        src = s1
                    else:
                        if m >= 0:
                            # causal mask on the diagonal 128 block, in-place
                            nc.vector.tensor_add(sp[:, :128], sp[:, :128],
                                                 tri[:])
                        src = sp
                    nc.scalar.activation(
                        p[:, :nv], src[:, :nv],
                        mybir.ActivationFunctionType.Exp,
                        bias=ab_sb[:, col:col + 1],
                        scale=AEXP,
                    )
                    pend.append((h, j, p, off, nv))

                def emit_pv():
                    h, j, p, off, nv = pend.pop(0)
                    j0 = JMIN[h][s]
                    if j == j0:
                        o_ps = ps_o.tile([128, W], F32, tag="ops",
                                         name=f"ops{h}")
                        acc = daccpool.tile([128, W],
                                            BF16 if SLOT_BF[h] else F16,
                                            tag="dacc", name=f"dacc{h}_{s}")
                        state[h] = (o_ps, acc)
                    o_ps, acc = state[h]
                    if j < 8:
                        jj = j - PK0[h]
                        vt = pv_sb[h][:, jj * 128:(jj + 1) * 128]
                    elif SLOT_BF[h]:
                        vt = vNbf[j - 8][:, VNB_IDX[h], :]
                    else:
                        vt = vN16[j - 8][:, VN_IDX[h], :]
                    nc.tensor.matmul(
                        o_ps[:, off:], vt, p[:, :nv],
                        start=(j == j0), stop=(j == nvis - 1),
                    )
                    # denominator accumulation on DVE (fp16, 2x mode)
                    if j == j0:
                        if off:
                            nc.vector.memset(acc[:, :off], 0.0)
                        nc.vector.tensor_copy(acc[:, off:], p[:, :nv])
                    else:
                        nc.vector.tensor_add(
                            acc[:, off:], acc[:, off:], p[:, :nv])
                    if j == nvis - 1:
                        # denominator + normalize (d reuses a scores psum slot)
                        # ones16 = 1/SAT so bb = SAT/den and at16 = at*SAT.
                        d_ps = ps_s.tile([128, W], F32, tag="sps", name="dps")
                        nc.tensor.matmul(
                            d_ps[0:1, :],
                            (onesbf if SLOT_BF[h] else ones16)[:], acc[:],
                            start=True, stop=True,
                        )
                        denr = smallpool.tile([1, W], F32, tag="denr",
                                              name="denr")
                        nc.vector.reciprocal(denr[:], d_ps[0:1, :])
                        bb = s1pool.tile([128, W], F32, tag="bb", bufs=2,
                                         name="bb")
                        nc.gpsimd.partition_broadcast(bb[:], denr[:])
                        at16 = atpool.tile([128, W], F16, tag="at16",
                                           name=f"at16_{h}_{s}")
                        nc.vector.tensor_mul(at16[:], o_ps[:], bb[:])
                        # hi/lo fp8 split for the o_proj DoubleRow matmuls
                        hp, r = h // 2, h % 2
                        if (hp, s) not in at8_tiles:
                            ath = atpool.tile([128, 2, W], F8, tag=f"a8h{hp}",
                                              name=f"a8h{hp}_{s}")
                            atl = atpool.tile([128, 2, W], F8, tag=f"a8l{hp}",
                                              name=f"a8l{hp}_{s}")
                            at8_tiles[(hp, s)] = (ath, atl)
                        ath, atl = at8_tiles[(hp, s)]
                        nc.scalar.activation(
                            ath[:, r, :], at16[:],
                            mybir.ActivationFunctionType.Copy)
                        nc.vector.tensor_sub(atl[:, r, :], at16[:], ath[:, r, :])

                for idx, (h, j) in enumerate(tiles):
                    emit_scores(h, j)
                    # interleave filler work proportionally
                    want = (idx + 1) * nfill // ntiles
                    while filled < want:
                        fillers[filled]()
                        filled += 1
                    if len(pend) >= DEPTH:
                        emit_pv()
                while pend:
                    emit_pv()
                while filled < nfill:
                    fillers[filled]()
                    filled += 1

            # ---- pipelined emission order ----
            for g in qkv_groups(0):
                g()
            # zero the r=1 plane of kT8 once (DVE idles here); scores
            # matmuls contract over [128, 2] with an all-zero r=1 plane
            # (moving-side garbage multiplies the zeros, so it's inert).
            for h in range(HPC):
                nc.gpsimd.memset(kT8[h][:, 1, :], 0.0)
            for g in qkv_groups(1):
                g()
            # attention tables ride the SP queue behind both x-slab sets
            # (needed only when attention_all(0) starts)
            deferred_init()
            # x slabs prefetched one phase ahead of their filler groups
            g2 = qkv_groups(2)
            g2[0]()
            attention_all(0, g2[1:])
            g3 = qkv_groups(3)
            g3[0]()
            op0, _ = oproj_groups(0)
            attention_all(1, g3[1:] + op0)
            op1, _ = oproj_groups(1)
            attention_all(2, op1)
            op2, _ = oproj_groups(2)
            op3, op3_pre = oproj_groups(3)
            # prefetch the tail's first o_proj weight tiles late in the last
            # attention phase (after op2's n=6 group so the opw pool buffer
            # it rotates into is already drained -- no SP-queue block)
            attention_all(3, op2[:25] + [op3_pre] + op2[25:])
            for g in op3:
                g()

    return nc


_CACHE = {}


def _build():
    if "nc" not in _CACHE:
        nc = bacc.Bacc(
            trn_type="TRN2", target_bir_lowering=False, debug=False,
            num_devices=NCORES,
        )
        _emit(nc)
        nc.compile()
        _CACHE["nc"] = nc
    return _CACHE["nc"]


def _pair8(a):
    """[D, F] -> fp8 pair layout [NPAIR, 128, 2, F]."""
    Dd, F = a.shape
    return np.ascontiguousarray(
        a.reshape(NPAIR, 2, 128, F).transpose(0, 2, 1, 3)
    ).astype(NPF8)


def _pair8_pm(a):
    """[D, F] -> fp8 partition-major pair layout [128, NPAIR, 2, F]."""
    Dd, F = a.shape
    return np.ascontiguousarray(
        a.reshape(NPAIR, 2, 128, F).transpose(2, 0, 1, 3)
    ).astype(NPF8)


def _host_prep(hidden_states, past_key, past_value, W_pack_w, o_proj_w):
    x = np.asarray(hidden_states, np.float32).reshape(S, D)
    pk = np.asarray(past_key, np.float32).reshape(H, HIST, HD)
    pv = np.asarray(past_value, np.float32).reshape(H, HIST, HD)
    Wp = np.asarray(W_pack_w, np.float32)
    Wo = np.asarray(o_proj_w, np.float32)
    slopes = _alibi_slopes(H)

    xT = np.ascontiguousarray(x.T)
    xs = xT * SX
    xh = xs.astype(NPF8).astype(np.float32)
    x8 = _pair8_pm(xh)                              # hi (exactly representable)
    xl8 = _pair8_pm(xs - xh)                        # lo residual

    scale = np.float32(1.0 / np.sqrt(HD))
    kk = np.arange(128, dtype=np.float32)
    ii = np.arange(W, dtype=np.float32)

    in_maps = []
    for c in range(NCORES):
        heads = [8 * i + c for i in range(HPC)]
        rsel = np.concatenate(
            [np.arange(hh * HD, (hh + 1) * HD) for hh in heads])
        Wq = Wp[rsel] * scale
        Wk = Wp[D + rsel]
        Wv = Wp[2 * D + rsel]
        Wqk = np.concatenate([Wq * SWQ, Wk * SWK], 0).T  # [D, 1024]
        w8 = _pair8(Wqk)
        wvs = Wv.T * SWV                                 # [D, FPC]
        wvh = wvs.astype(NPF8).astype(np.float32)
        wv8h = _pair8(wvh)
        wv8l = _pair8(wvs - wvh)
        # flat truncated past-K in fp8 [128, 2, n*128] per slot, r=1 zeroed
        pk8_segs = []
        for h in range(HPC):
            seg = np.zeros((128, 2, NPK[h] * 128), np.float32)
            seg[:, 0, :] = pk[heads[h]][PK0[h] * 128:, :].T * SKC
            pk8_segs.append(seg.reshape(128, -1))
        pk8 = np.ascontiguousarray(
            np.concatenate(pk8_segs, axis=1)).astype(NPF8)
        pv16c = np.ascontiguousarray(np.concatenate(
            [pv[heads[h]][PK0[h] * 128:, :] for h in (0, 3)], axis=0
        )).astype(np.float16)
        pvbc = np.ascontiguousarray(np.concatenate(
            [pv[heads[h]][PK0[h] * 128:, :] for h in (1, 2)], axis=0
        )).astype(NPBF)
        # o_proj weights, hi/lo fp8, head-pair DoubleRow layout
        # [n, p, hp, r, W] (pre-sliced by 512-wide output group)
        wos = np.zeros((128, 2, 2, D), np.float32)
        for hh in range(HPC):
            wos[:, hh // 2, hh % 2, :] = (
                Wo[:, heads[hh] * HD:(heads[hh] + 1) * HD].T * SWO)
        wos = np.ascontiguousarray(
            wos.reshape(128, 2, 2, 8, W).transpose(3, 0, 1, 2, 4))
        woh8 = wos.astype(NPF8)
        wol8 = (wos - woh8.astype(np.float32)).astype(NPF8)
        sl = slopes[heads]

        ab = np.zeros((HPC, NST, NKT, 128), np.float32)
        for h in range(HPC):
            for s in range(NST):
                for j in range(NKT):
                    ab[h, s, j] = sl[h] * (128 * j + kk - HIST - W * s)
                    m = j - (8 + 4 * s)
                    if h == 0 and m >= 0:
                        # diag0 tile carries -sl*c; shift the per-key bias
                        # so the pair reproduces sl*(k_pos - q_pos)
                        ab[h, s, j] -= sl[h] * 128 * m
            if CQ[h] is not None:
                # per-query recentering constant (softmax-invariant)
                ab[h] -= sl[h] * CQ[h]
        ab_sb = np.ascontiguousarray(
            ab.reshape(HPC * NST * NKT, 128).T
        )

        mkpat = np.where(ii[None, :128] >= kk[:, None], 0.0,
                         NEG).astype(np.float32)
        dvb = np.zeros((2, 128, W), np.float32)
        dvb[0, :, :128] = mkpat
        dvb[0] += (np.float32(RS) * -sl[0] * ii)[None, :]
        dvb[1] = (np.float32(RS) * -sl[0] * ii)[None, :]
        mk = mkpat[None]
        in_maps.append({
            "x8": x8, "xl8": xl8, "w8": w8, "wv8h": wv8h, "wv8l": wv8l,
            "pk8": pk8, "pv16": pv16c, "pvb": pvbc,
            "woh8": woh8, "wol8": wol8,
            "abias": ab_sb, "dvb": dvb, "mk": mk,
        })
    return in_maps


def kernel(hidden_states, past_key, past_value, W_pack_w, o_proj_w):
    nc = _build()
    in_maps = _host_prep(hidden_states, past_key, past_value, W_pack_w, o_proj_w)
    res = run_bass_kernel_spmd(nc, in_maps, list(range(NCORES)))
    out = np.zeros((S, D), np.float64)
    for c in range(NCORES):
        out += res.results[c]["outp"].astype(np.float64)
    return out.astype(np.float32).reshape(B, S, D)



# revision 72
# speedup vs baseline: 1.0340x; 1.0096x over previous
"""Trainium2 Bass kernel for nn_BaichuanAttention (ALiBi attention + KV cache).

Head-parallel across 8 NeuronCores (4 heads/core). Per core:
  - Q/K projection: fp8e4 DoubleRow matmuls (256-deep contraction at
    0.5 cycles/row); weights resident in SBUF. Scores insensitive to
    fp8 rounding (ALiBi dominates the softmax).
  - V projection: 3-term hi/lo fp8 DoubleRow (x = xh + xl, Wv = wh + wl,
    v ~= wh@xh + wh@xl + wl@xh) giving ~fp16 accuracy at 0.75x the fp16
    matmul cost.
  - attention with transposed scores [keys, queries]:
      * scores via fp8 DoubleRow matmuls (q/k requantized to fp8; the
        stationary r=1 plane is zero so the 256-deep DR contraction
        reduces to the real 128-deep hd one at 0.5 cycles/row)
      * ALiBi per-key term folded into the ACT exp() per-partition bias
      * ALiBi per-query term + causal mask applied via DVE adds
      * softmax denominator via ones-matmul on the PE (ones=1/SAT so the
        normalized attention output comes out pre-scaled by SAT)
  - PV accumulation (fp16), normalize via outer-product broadcast
  - o_proj: 3-term hi/lo fp8 DoubleRow over head pairs
    (ath@woh + atl@woh + ath@wol), psum descaled by 1/(SAT*SWO)
Host: shard/transpose/cast/scale inputs, sum fp16 o_proj partials.
"""
import os
import sys

import numpy as np

for _p in ("/opt/trn_rl_repo",):
    if os.path.isdir(_p) and _p not in sys.path:
        sys.path.insert(0, _p)

import ml_dtypes
import concourse.bass as bass
import concourse.mybir as mybir
import concourse.tile as tile
from concourse import bacc
from concourse.bass_utils import run_bass_kernel_spmd
from concourse.masks import make_identity

F32 = mybir.dt.float32
F16 = mybir.dt.float16
BF16 = mybir.dt.bfloat16
F8 = mybir.dt.float8e4
NPF8 = ml_dtypes.float8_e4m3
NPBF = ml_dtypes.bfloat16

B, S, D, H, HD, HIST = 1, 2048, 4096, 32, 128, 1024
T = HIST + S
NCORES = 8
HPC = H // NCORES          # heads per core
FPC = HPC * HD             # 512 features per core per section
NST = S // 512             # 4 query supertiles
W = 512                    # supertile width
NKT = T // 128             # 24 key tiles
NKC = D // 128             # 32 contraction chunks for QKV
NPAIR = NKC // 2           # 16 DoubleRow contraction pairs
NEG = -1.0e30

SX = 128.0                 # x fp8 scale
SWQ = 2048.0               # Wq (incl 1/sqrt(hd)) fp8 scale
SWK = 128.0                # Wk fp8 scale
SWV = 128.0                # Wv fp8 scale
SV_INV = 1.0 / (SX * SWV)  # V psum descale
# q/k requantized to fp8 for DoubleRow scores (zero-padded r=1 plane);
# uniform scale for past and new keys.
SQC = 8192.0               # q fp8 scale
SKC = 32.0                 # k fp8 scale (past |k|~5.5 sigma * 32 < 240)
QF = SQC / (SX * SWQ)      # q psum -> fp8 rescale
KF = SKC / (SX * SWK)      # k psum -> fp8 rescale
RS = SQC * SKC             # scores psum scale (all tiles)
AEXP = 1.0 / RS            # ACT exp input descale
SWO = 128.0                # Wo fp8 scale
SAT = 32.0                 # attn-out fp8 scale (folded into 1/den via ones)
SO_INV = 1.0 / (SAT * SWO)  # o_proj psum descale


def _alibi_slopes(n_heads: int) -> np.ndarray:
    def pow2_slopes(m):
        start = 2.0 ** (-(2.0 ** -(np.log2(m) - 3)))
        return start * (start ** np.arange(m))
    if np.log2(n_heads).is_integer():
        return pow2_slopes(n_heads).astype(np.float32)
    m = 2 ** int(np.floor(np.log2(n_heads)))
    base = pow2_slopes(m)
    extra = pow2_slopes(2 * m)[0::2][: n_heads - m]
    return np.concatenate([base, extra]).astype(np.float32)


# --- ALiBi window truncation ---------------------------------------------
# Keys further than MARGIN/slope behind a query contribute exp(-MARGIN)
# relative weight — drop their tiles. Heads are distributed so core c gets
# heads {c, 8+c, 16+c, 24+c} (slot i = head 8i+c); each slot's window is
# sized for the *smallest* slope in its group, so every core runs an
# identical instruction stream.
MARGIN = 11.0
_SLOPES_ALL = _alibi_slopes(H)
JMIN = []
for _i in range(HPC):
    _win = MARGIN / float(_SLOPES_ALL[8 * _i + 7])
    JMIN.append([
        max(0, min(12 + 4 * _s, int((HIST + W * _s - _win) // 128)))
        for _s in range(NST)
    ])
# past-key/value tiles actually reachable per slot (j in [PK0[h], 8))
PK0 = [min(JMIN[_i][0], 8) for _i in range(HPC)]
NPK = [8 - PK0[_i] for _i in range(HPC)]
PKOFF = [sum(NPK[:_i]) for _i in range(HPC)]   # segment offsets, in tiles
NPKT = sum(NPK)

# --- per-slot softmax numerics -------------------------------------------
# Softmax is invariant to any per-query constant added to all visible
# scores; the per-query ALiBi term -sl*q_pos is only needed to avoid
# overflow.  Per slot we use the cheapest scheme the slope allows:
#   slot 0 (steep): exact -sl*i row bias via DVE add, p fp16
#   slots 1,2:      constant recenter C=sl*256, p bf16 (range to e^54)
#   slot 3 (flat):  constant recenter C=sl*128, p fp16 (max e^5)
# so slots 1-3 need NO DVE op on non-diagonal tiles (ACT reads psum).
CQ = [None, 256.0, 256.0, 128.0]
SLOT_BF = [False, True, True, False]
# fp16 slots 0,3 share vN16 (r: 0->slot0, 1->slot3); bf16 slots share vNbf
VN_IDX = {0: 0, 3: 1}
VNB_IDX = {1: 0, 2: 1}
NPK16 = NPK[0] + NPK[3]
NPKBF = NPK[1] + NPK[2]


def _emit(nc):
    """Emit the whole per-core program under a TileContext."""
    x8_d = nc.dram_tensor("x8", [128, NPAIR, 2, S], F8, kind="ExternalInput").ap()
    xl8_d = nc.dram_tensor("xl8", [128, NPAIR, 2, S], F8, kind="ExternalInput").ap()
    w8_d = nc.dram_tensor("w8", [NPAIR, 128, 2, 1024], F8, kind="ExternalInput").ap()
    wv8h_d = nc.dram_tensor("wv8h", [NPAIR, 128, 2, FPC], F8, kind="ExternalInput").ap()
    wv8l_d = nc.dram_tensor("wv8l", [NPAIR, 128, 2, FPC], F8, kind="ExternalInput").ap()
    pk8_d = nc.dram_tensor("pk8", [128, 2 * NPKT * 128], F8, kind="ExternalInput").ap()
    pv16_d = nc.dram_tensor("pv16", [NPK16 * 128, HD], F16, kind="ExternalInput").ap()
    pvb_d = nc.dram_tensor("pvb", [NPKBF * 128, HD], BF16, kind="ExternalInput").ap()
    woh_d = nc.dram_tensor("woh8", [8, 128, 2, 2, W], F8, kind="ExternalInput").ap()
    wol_d = nc.dram_tensor("wol8", [8, 128, 2, 2, W], F8, kind="ExternalInput").ap()
    ab_d = nc.dram_tensor("abias", [128, HPC * NST * NKT], F32, kind="ExternalInput").ap()
    # dvb[0] = slot0 diag add (tri + -sl*c*RS), dvb[1] = slot0 row bias
    dvb_d = nc.dram_tensor("dvb", [2, 128, W], F32, kind="ExternalInput").ap()
    mk_d = nc.dram_tensor("mk", [1, 128, 128], F32, kind="ExternalInput").ap()
    out_d = nc.dram_tensor("outp", [S, D], F16, kind="ExternalOutput").ap()

    with tile.TileContext(nc) as tc:
        with (
            tc.tile_pool(name="persist", bufs=1) as pers,
            tc.tile_pool(name="x8slab", bufs=2) as x8pool,
            tc.tile_pool(name="xl8slab", bufs=2) as xl8pool,
            tc.tile_pool(name="qp", bufs=2) as qpool,
            tc.tile_pool(name="opwp", bufs=3) as opwpool,
            tc.tile_pool(name="s1", bufs=3) as s1pool,
            tc.tile_pool(name="pp", bufs=7) as ppool,
            tc.tile_pool(name="dac", bufs=2) as daccpool,
            tc.tile_pool(name="small", bufs=1) as smallpool,
            tc.tile_pool(name="ob", bufs=3) as obpool,
            tc.tile_pool(name="at", bufs=2) as atpool,
            tc.tile_pool(name="ps_qkv", bufs=2, space="PSUM") as ps_qkv,
            tc.tile_pool(name="ps_s", bufs=2, space="PSUM") as ps_s,
            tc.tile_pool(name="ps_o", bufs=2, space="PSUM") as ps_o,
            tc.tile_pool(name="ps_sh", bufs=2, space="PSUM") as ps_sh,
        ):
            # ---- persistent SBUF tensors ----
            # k in fp8 for DoubleRow scores: r=1 plane kept zero so the
            # 256-deep DR contraction reduces to the real 128-deep one.
            kT8 = [pers.tile([128, 2, S], F8, tag=f"kT{h}", bufs=1, name=f"kT{h}") for h in range(HPC)]
            # new-v per 128-key chunk: fp16 slots {0,3} and bf16 slots {1,2}
            vN16 = [pers.tile([128, 2, HD], F16, tag=f"vN{t}", bufs=1, name=f"vN{t}")
                    for t in range(S // 128)]
            vNbf = [pers.tile([128, 2, HD], BF16, tag=f"vB{t}", bufs=1, name=f"vB{t}")
                    for t in range(S // 128)]
            pk8_sb = [pers.tile([128, 2, NPK[h] * 128], F8, tag=f"pk{h}", bufs=1, name=f"pk{h}") for h in range(HPC)]
            pv_sb = [pers.tile([128, NPK[h] * 128],
                               BF16 if SLOT_BF[h] else F16,
                               tag=f"pvs{h}", bufs=1, name=f"pvs{h}")
                     for h in range(HPC)]
            at8_tiles = {}
            q_tiles = {}
            ab_sb = pers.tile([128, HPC * NST * NKT], F32, tag="abias", bufs=1)
            ones16 = pers.tile([128, 1], F16, tag="ones16", bufs=1)
            onesbf = pers.tile([128, 1], BF16, tag="onesbf", bufs=1)
            # resident fp8 weights
            w8 = [pers.tile([128, 2, 1024], F8, tag=f"w8_{c}", bufs=1, name=f"w8_{c}")
                  for c in range(NPAIR)]
            wv8h = [pers.tile([128, 2, FPC], F8, tag=f"wvh{c}", bufs=1, name=f"wvh{c}")
                    for c in range(NPAIR)]
            wv8l = [pers.tile([128, 2, FPC], F8, tag=f"wvl{c}", bufs=1, name=f"wvl{c}")
                    for c in range(NPAIR)]

            nc.any.memset(ones16[:], 1.0 / SAT)
            nc.any.memset(onesbf[:], 1.0 / SAT)
            # w8 is the startup-critical load: split it over the Pool and
            # ACT queues.  wv8l rides Pool behind it; everything else is
            # deferred onto the SP queue after supertile 0's x slabs (see
            # deferred_init below) so the ACT/DVE queues start empty.
            for c in range(NPAIR):
                # alternate queues so w8 arrival keeps pace with consumption
                (nc.gpsimd if c % 2 == 0 else nc.scalar).dma_start(
                    w8[c][:], w8_d[c])
            for c in range(NPAIR):
                nc.gpsimd.dma_start(wv8l[c][:], wv8l_d[c])
                nc.scalar.dma_start(wv8h[c][:], wv8h_d[c])

            # slot0 diag/row bias tiles + tri mask
            diag0 = pers.tile([128, W], F32, tag="diag0", bufs=1)
            rowt0 = pers.tile([128, W], F32, tag="rowt0", bufs=1)
            tri = pers.tile([128, 128], F32, tag="tri", bufs=1)

            def deferred_init():
                nc.sync.dma_start(ab_sb[:], ab_d[:])
                for h in range(HPC):
                    nc.sync.dma_start(
                        pk8_sb[h][:].rearrange("p r n -> p (r n)"),
                        pk8_d[:, 2 * PKOFF[h] * 128:
                              2 * (PKOFF[h] + NPK[h]) * 128])
                for h, dsrc, off in ((0, pv16_d, 0), (3, pv16_d, NPK[0]),
                                     (1, pvb_d, 0), (2, pvb_d, NPK[1])):
                    nc.sync.dma_start(
                        pv_sb[h].rearrange("p (c d) -> p c d", c=NPK[h]),
                        dsrc[off * 128:(off + NPK[h]) * 128]
                        .rearrange("(c p) d -> p c d", p=128),
                    )
                nc.sync.dma_start(diag0[:], dvb_d[0])
                nc.sync.dma_start(rowt0[:], dvb_d[1])
                nc.sync.dma_start(tri[:], mk_d[0])

            def qkv_groups(sc):
                """Return filler closures for supertile sc's QKV projection:
                one x-load group, 8 Q/K feature-chunk groups, 4 V groups."""
                x8t = [None] * 2
                xl8t = [None] * 2

                def xload():
                    for g in range(2):
                        t = x8pool.tile([128, 8, 2, W], F8, tag="x8",
                                        name=f"x8_{sc}_{g}")
                        # split slab DMAs so the first QK matmuls can start
                        # before the whole slab lands (finest for supertile 0
                        # where nothing else hides the latency)
                        nsp = 4 if sc == 0 else 2
                        for hh in range(nsp):
                            cw = 8 // nsp
                            c0 = g * 8 + hh * cw
                            nc.sync.dma_start(
                                t[:, hh * cw:(hh + 1) * cw],
                                x8_d[:, c0:c0 + cw, :, sc * W:(sc + 1) * W],
                            )
                        x8t[g] = t
                    for g in range(2):
                        tl = xl8pool.tile([128, 8, 2, W], F8, tag="xl8",
                                          name=f"xl8_{sc}_{g}")
                        nc.sync.dma_start(
                            tl[:],
                            xl8_d[:, g * 8:(g + 1) * 8, :, sc * W:(sc + 1) * W],
                        )
                        xl8t[g] = tl

                def qk_group(fc):
                    def emit():
                        psum = ps_qkv.tile([128, W], F32, tag="qkvps", name="qkvps")
                        for c in range(NPAIR):
                            nc.tensor.matmul(
                                psum[:],
                                w8[c][:, :, fc * 128:(fc + 1) * 128],
                                x8t[c // 8][:, c % 8, :, :],
                                start=(c == 0), stop=(c == NPAIR - 1),
                                perf_mode=mybir.MatmulPerfMode.DoubleRow,
                            )
                        if fc < 4:
                            qt = qpool.tile([128, 2, W], F8, tag=f"q{fc}",
                                            name=f"q{fc}_{sc}")
                            q_tiles[(fc, sc)] = qt
                            if sc < 2:
                                # first use of this pool buffer: make the
                                # r=1 plane finite (contents are irrelevant
                                # -- the stationary r=1 plane is zero)
                                nc.vector.memset(qt[:, 1, :], 0.0)
                            nc.vector.tensor_scalar_mul(
                                qt[:, 0, :], psum[:], QF)
                        else:
                            nc.vector.tensor_scalar_mul(
                                kT8[fc - 4][:, 0, sc * W:(sc + 1) * W],
                                psum[:], KF)
                    return emit

                def v_group(t4):
                    """V for key-chunk t4 of this supertile, directly in
                    [keys, 4*hd] layout: stationary = x pairs, moving = Wv."""
                    def emit():
                        psum = ps_qkv.tile([128, W], F32, tag="qkvps", name="qkvps")
                        nmm = 3 * NPAIR
                        i = 0
                        ksl = slice(t4 * 128, (t4 + 1) * 128)
                        for c in range(NPAIR):
                            xs = x8t[c // 8][:, c % 8, :, ksl]
                            xls = xl8t[c // 8][:, c % 8, :, ksl]
                            for (lhs, rhs) in (
                                (xs, wv8h[c]),
                                (xls, wv8h[c]),
                                (xs, wv8l[c]),
                            ):
                                nc.tensor.matmul(
                                    psum[:], lhs, rhs[:],
                                    start=(i == 0), stop=(i == nmm - 1),
                                    perf_mode=mybir.MatmulPerfMode.DoubleRow,
                                )
                                i += 1
                        t = sc * 4 + t4
                        # psum features are slot-major: [h0|h1|h2|h3]*128
                        nc.vector.tensor_scalar_mul(
                            vN16[t][:],
                            psum[:].rearrange("p (g d) -> p g d", g=4)
                            [:, 0::3, :],
                            SV_INV)
                        nc.vector.tensor_scalar_mul(
                            vNbf[t][:].rearrange("p r d -> p (r d)"),
                            psum[:, 128:384], SV_INV)
                    return emit

                return ([xload] + [qk_group(fc) for fc in range(8)]
                        + [v_group(t4) for t4 in range(4)])

            def oproj_groups(s):
                """o_proj partial rows for supertile s (32 closures; 3-term
                hi/lo fp8 DoubleRow over head pairs; batched weight loads
                prefetched one n-group ahead, 2-batched output stores on the
                Pool DMA queue)."""
                out = []
                opn = {}

                def wload(n):
                    th = opwpool.tile([128, 2, 2, W], F8, tag="opwh",
                                      name=f"opwh{n}_{s}")
                    nc.sync.dma_start(th[:], woh_d[n])
                    tl = opwpool.tile([128, 2, 2, W], F8, tag="opwl",
                                      name=f"opwl{n}_{s}")
                    nc.sync.dma_start(tl[:], wol_d[n])
                    opn[n] = (th, tl)

                tail = (s == 3)
                for n in range(8):
                    obt = [None]
                    for m4 in range(4):
                        def grp(n=n, m4=m4, obt=obt):
                            if n == 0 and m4 == 0 and 0 not in opn:
                                wload(0)
                            m = s * 4 + m4
                            if tail:
                                # attention is done: rotate over all psum
                                # pools so the obt drain never gates the PE
                                pspool = (ps_sh, ps_qkv, ps_s)[
                                    (n * 4 + m4) % 3]
                                psum = pspool.tile(
                                    [128, W], F32,
                                    tag={id(ps_sh): "sh", id(ps_qkv): "qkvps",
                                         id(ps_s): "sps"}[id(pspool)],
                                    name="shps")
                            else:
                                psum = ps_sh.tile([128, W], F32, tag="sh",
                                                  name="shps")
                            woh, wol = opn[n]
                            msl = slice(m4 * 128, (m4 + 1) * 128)
                            i = 0
                            for hp in range(2):
                                ath, atl = at8_tiles[(hp, s)]
                                for (lhs, rhs) in (
                                    (ath, woh), (atl, woh), (ath, wol),
                                ):
                                    nc.tensor.matmul(
                                        psum[:],
                                        lhs[:, :, msl],
                                        rhs[:, hp, :, :],
                                        start=(i == 0), stop=(i == 5),
                                        perf_mode=mybir.MatmulPerfMode.DoubleRow,
                                    )
                                    i += 1
                            if m4 == 0 and n + 1 < 8:
                                wload(n + 1)

                            if m4 % 2 == 0:
                                obt[0] = obpool.tile([128, 2, W], F16,
                                                     tag="ob", name="ob")
                            # alternate engines so the psum drain pipelines
                            if m4 % 2 == 0:
                                nc.vector.tensor_scalar_mul(
                                    obt[0][:, 0, :], psum[:], SO_INV)
                            else:
                                nc.scalar.activation(
                                    obt[0][:, 1, :], psum[:],
                                    mybir.ActivationFunctionType.Copy,
                                    scale=SO_INV)
                            if m4 % 2 == 1:
                                # tail: three store queues so the final drain
                                # pipelines (ACT/SP are hwdge engines too)
                                eng = ((nc.gpsimd, nc.scalar, nc.sync)
                                       [(n * 2 + m4 // 2) % 3]
                                       if tail else nc.gpsimd)
                                eng.dma_start(
                                    out_d[(m - 1) * 128:(m + 1) * 128,
                                          n * W:(n + 1) * W]
                                    .rearrange("(two p) f -> p two f", p=128),
                                    obt[0][:],
                                )
                        out.append(grp)
                return out, (lambda: wload(0) if 0 not in opn else None)

            def attention_all(s, fillers):
                """All heads for supertile s, software-pipelined (scores run
                DEPTH tiles ahead of PV) with filler groups interleaved to
                keep the PE busy during the add->exp latency chain."""
                nvis = 12 + 4 * s
                tiles = [(h, j) for h in range(HPC)
                         for j in range(JMIN[h][s], nvis)]
                DEPTH = 2 if s == 1 else 6
                ntiles = len(tiles)
                nfill = len(fillers)
                filled = 0
                state = {}   # h -> (o_ps, acc)
                pend = []    # [(h, j, p, off, nv)]
                norm_pend = []  # [(h, emitted_at)] deferred normalizations
                ecnt = 0
                scnt = 0

                def emit_scores(h, j):
                    nonlocal scnt
                    m = j - (8 + 4 * s)
                    off = 0 if m < 0 else 128 * m
                    nv = W - off
                    if s >= 2 and scnt % 2 == 1:
                        sp = ps_qkv.tile([128, W], F32, tag="qkvps", name="qkvps")
                    elif s == 0 and scnt % 2 == 1:
                        sp = ps_sh.tile([128, W], F32, tag="sh", name="shps")
                    else:
                        sp = ps_s.tile([128, W], F32, tag="sps", name="sps")
                    scnt += 1
                    if j < 8:
                        jj = j - PK0[h]
                        kt = pk8_sb[h][:, :, jj * 128:(jj + 1) * 128]
                    else:
                        kt = kT8[h][:, :, (j - 8) * 128:(j - 7) * 128]
                    nc.tensor.matmul(
                        sp[:, :nv], kt,
                        q_tiles[(h, s)][:, :, off:],
                        start=True, stop=True,
                        perf_mode=mybir.MatmulPerfMode.DoubleRow,
                    )
                    p = ppool.tile([128, W], BF16 if SLOT_BF[h] else F16,
                                   tag="p", name="p")
                    col = (h * NST + s) * NKT + j
                    if h == 0:
                        # steep slope: exact per-query row bias (+tri on diag)
                        s1 = s1pool.tile([128, W], F32, tag="s1", bufs=2,
                                             name="s1")
                        rt = diag0 if m >= 0 else rowt0
                        nc.vector.tensor_add(s1[:, :nv], sp[:, :nv],
                                             rt[:, :nv])
                        src = s1
                    else:
                        if m >= 0:
                            # causal mask on the diagonal 128 block, in-place
                            nc.vector.tensor_add(sp[:, :128], sp[:, :128],
                                                 tri[:])
                        src = sp
                    nc.scalar.activation(
                        p[:, :nv], src[:, :nv],
                        mybir.ActivationFunctionType.Exp,
                        bias=ab_sb[:, col:col + 1],
                        scale=AEXP,
                    )
                    pend.append((h, j, p, off, nv))

                def emit_pv():
                    h, j, p, off, nv = pend.pop(0)
                    j0 = JMIN[h][s]
                    if j == j0:
                        o_ps = ps_o.tile([128, W], F32, tag="ops",
                                         name=f"ops{h}")
                        acc = daccpool.tile([128, W],
                                            BF16 if SLOT_BF[h] else F16,
                                            tag="dacc", name=f"dacc{h}_{s}")
                        state[h] = (o_ps, acc)
                    o_ps, acc = state[h]
                    if j < 8:
                        jj = j - PK0[h]
                        vt = pv_sb[h][:, jj * 128:(jj + 1) * 128]
                    elif SLOT_BF[h]:
                        vt = vNbf[j - 8][:, VNB_IDX[h], :]
                    else:
                        vt = vN16[j - 8][:, VN_IDX[h], :]
                    nc.tensor.matmul(
                        o_ps[:, off:], vt, p[:, :nv],
                        start=(j == j0), stop=(j == nvis - 1),
                    )
                    # denominator accumulation on DVE (fp16, 2x mode)
                    if j == j0:
                        if off:
                            nc.vector.memset(acc[:, :off], 0.0)
                        nc.vector.tensor_copy(acc[:, off:], p[:, :nv])
                    else:
                        nc.vector.tensor_add(
                            acc[:, off:], acc[:, off:], p[:, :nv])
                    if j == nvis - 1:
                        norm_pend.append((h, ecnt))

                def do_normalize(h):
                        o_ps, acc = state[h]
                        # denominator + normalize (d reuses a scores psum slot)
                        # ones16 = 1/SAT so bb = SAT/den and at16 = at*SAT.
                        d_ps = ps_s.tile([128, W], F32, tag="sps", name="dps")
                        nc.tensor.matmul(
                            d_ps[0:1, :],
                            (onesbf if SLOT_BF[h] else ones16)[:], acc[:],
                            start=True, stop=True,
                        )
                        denr = smallpool.tile([1, W], F32, tag="denr",
                                              name="denr")
                        nc.vector.reciprocal(denr[:], d_ps[0:1, :])
                        bb = s1pool.tile([128, W], F32, tag="bb", bufs=2,
                                         name="bb")
                        nc.gpsimd.partition_broadcast(bb[:], denr[:])
                        at16 = atpool.tile([128, W], F16, tag="at16",
                                           name=f"at16_{h}_{s}")
                        nc.vector.tensor_mul(at16[:], o_ps[:], bb[:])
                        # hi/lo fp8 split for the o_proj DoubleRow matmuls
                        hp, r = h // 2, h % 2
                        if (hp, s) not in at8_tiles:
                            ath = atpool.tile([128, 2, W], F8, tag=f"a8h{hp}",
                                              name=f"a8h{hp}_{s}")
                            atl = atpool.tile([128, 2, W], F8, tag=f"a8l{hp}",
                                              name=f"a8l{hp}_{s}")
                            at8_tiles[(hp, s)] = (ath, atl)
                        ath, atl = at8_tiles[(hp, s)]
                        nc.scalar.activation(
                            ath[:, r, :], at16[:],
                            mybir.ActivationFunctionType.Copy)
                        nc.vector.tensor_sub(atl[:, r, :], at16[:], ath[:, r, :])

                for idx, (h, j) in enumerate(tiles):
                    emit_scores(h, j)
                    ecnt += 1
                    # normalizations deferred a few tiles so the PE stream
                    # doesn't stall on the DVE acc queue at head seams
                    while norm_pend and ecnt - norm_pend[0][1] >= 3:
                        do_normalize(norm_pend.pop(0)[0])
                    # interleave filler work proportionally
                    want = (idx + 1) * nfill // ntiles
                    while filled < want:
                        fillers[filled]()
                        filled += 1
                    if len(pend) >= DEPTH:
                        emit_pv()
                while pend:
                    emit_pv()
                while norm_pend:
                    do_normalize(norm_pend.pop(0)[0])
                while filled < nfill:
                    fillers[filled]()
                    filled += 1

            # ---- pipelined emission order ----
            for g in qkv_groups(0):
                g()
            # zero the r=1 plane of kT8 once (DVE idles here); scores
            # matmuls contract over [128, 2] with an all-zero r=1 plane
            # (moving-side garbage multiplies the zeros, so it's inert).
            for h in range(HPC):
                nc.gpsimd.memset(kT8[h][:, 1, :], 0.0)
            for g in qkv_groups(1):
                g()
            # attention tables ride the SP queue behind both x-slab sets
            # (needed only when attention_all(0) starts)
            deferred_init()
            # x slabs prefetched one phase ahead of their filler groups
            g2 = qkv_groups(2)
            g2[0]()
            attention_all(0, g2[1:])
            g3 = qkv_groups(3)
            g3[0]()
            op0, _ = oproj_groups(0)
            attention_all(1, g3[1:] + op0)
            op1, _ = oproj_groups(1)
            attention_all(2, op1)
            op2, _ = oproj_groups(2)
            op3, op3_pre = oproj_groups(3)
            # prefetch the tail's first o_proj weight tiles late in the last
            # attention phase (after op2's n=6 group so the opw pool buffer
            # it rotates into is already drained -- no SP-queue block)
            attention_all(3, op2[:25] + [op3_pre] + op2[25:])
            for g in op3:
                g()

    return nc


_CACHE = {}


def _build():
    if "nc" not in _CACHE:
        nc = bacc.Bacc(
            trn_type="TRN2", target_bir_lowering=False, debug=False,
            num_devices=NCORES,
        )
        _emit(nc)
        nc.compile()
        _CACHE["nc"] = nc
    return _CACHE["nc"]


def _pair8(a):
    """[D, F] -> fp8 pair layout [NPAIR, 128, 2, F]."""
    Dd, F = a.shape
    return np.ascontiguousarray(
        a.reshape(NPAIR, 2, 128, F).transpose(0, 2, 1, 3)
    ).astype(NPF8)


def _pair8_pm(a):
    """[D, F] -> fp8 partition-major pair layout [128, NPAIR, 2, F]."""
    Dd, F = a.shape
    return np.ascontiguousarray(
        a.reshape(NPAIR, 2, 128, F).transpose(2, 0, 1, 3)
    ).astype(NPF8)


def _host_prep(hidden_states, past_key, past_value, W_pack_w, o_proj_w):
    x = np.asarray(hidden_states, np.float32).reshape(S, D)
    pk = np.asarray(past_key, np.float32).reshape(H, HIST, HD)
    pv = np.asarray(past_value, np.float32).reshape(H, HIST, HD)
    Wp = np.asarray(W_pack_w, np.float32)
    Wo = np.asarray(o_proj_w, np.float32)
    slopes = _alibi_slopes(H)

    xT = np.ascontiguousarray(x.T)
    xs = xT * SX
    xh = xs.astype(NPF8).astype(np.float32)
    x8 = _pair8_pm(xh)                              # hi (exactly representable)
    xl8 = _pair8_pm(xs - xh)                        # lo residual

    scale = np.float32(1.0 / np.sqrt(HD))
    kk = np.arange(128, dtype=np.float32)
    ii = np.arange(W, dtype=np.float32)

    in_maps = []
    for c in range(NCORES):
        heads = [8 * i + c for i in range(HPC)]
        rsel = np.concatenate(
            [np.arange(hh * HD, (hh + 1) * HD) for hh in heads])
        Wq = Wp[rsel] * scale
        Wk = Wp[D + rsel]
        Wv = Wp[2 * D + rsel]
        Wqk = np.concatenate([Wq * SWQ, Wk * SWK], 0).T  # [D, 1024]
        w8 = _pair8(Wqk)
        wvs = Wv.T * SWV                                 # [D, FPC]
        wvh = wvs.astype(NPF8).astype(np.float32)
        wv8h = _pair8(wvh)
        wv8l = _pair8(wvs - wvh)
        # flat truncated past-K in fp8 [128, 2, n*128] per slot, r=1 zeroed
        pk8_segs = []
        for h in range(HPC):
            seg = np.zeros((128, 2, NPK[h] * 128), np.float32)
            seg[:, 0, :] = pk[heads[h]][PK0[h] * 128:, :].T * SKC
            pk8_segs.append(seg.reshape(128, -1))
        pk8 = np.ascontiguousarray(
            np.concatenate(pk8_segs, axis=1)).astype(NPF8)
        pv16c = np.ascontiguousarray(np.concatenate(
            [pv[heads[h]][PK0[h] * 128:, :] for h in (0, 3)], axis=0
        )).astype(np.float16)
        pvbc = np.ascontiguousarray(np.concatenate(
            [pv[heads[h]][PK0[h] * 128:, :] for h in (1, 2)], axis=0
        )).astype(NPBF)
        # o_proj weights, hi/lo fp8, head-pair DoubleRow layout
        # [n, p, hp, r, W] (pre-sliced by 512-wide output group)
        wos = np.zeros((128, 2, 2, D), np.float32)
        for hh in range(HPC):
            wos[:, hh // 2, hh % 2, :] = (
                Wo[:, heads[hh] * HD:(heads[hh] + 1) * HD].T * SWO)
        wos = np.ascontiguousarray(
            wos.reshape(128, 2, 2, 8, W).transpose(3, 0, 1, 2, 4))
        woh8 = wos.astype(NPF8)
        wol8 = (wos - woh8.astype(np.float32)).astype(NPF8)
        sl = slopes[heads]

        ab = np.zeros((HPC, NST, NKT, 128), np.float32)
        for h in range(HPC):
            for s in range(NST):
                for j in range(NKT):
                    ab[h, s, j] = sl[h] * (128 * j + kk - HIST - W * s)
                    m = j - (8 + 4 * s)
                    if h == 0 and m >= 0:
                        # diag0 tile carries -sl*c; shift the per-key bias
                        # so the pair reproduces sl*(k_pos - q_pos)
                        ab[h, s, j] -= sl[h] * 128 * m
            if CQ[h] is not None:
                # per-query recentering constant (softmax-invariant)
                ab[h] -= sl[h] * CQ[h]
        ab_sb = np.ascontiguousarray(
            ab.reshape(HPC * NST * NKT, 128).T
        )

        mkpat = np.where(ii[None, :128] >= kk[:, None], 0.0,
                         NEG).astype(np.float32)
        dvb = np.zeros((2, 128, W), np.float32)
        dvb[0, :, :128] = mkpat
        dvb[0] += (np.float32(RS) * -sl[0] * ii)[None, :]
        dvb[1] = (np.float32(RS) * -sl[0] * ii)[None, :]
        mk = mkpat[None]
        in_maps.append({
            "x8": x8, "xl8": xl8, "w8": w8, "wv8h": wv8h, "wv8l": wv8l,
            "pk8": pk8, "pv16": pv16c, "pvb": pvbc,
            "woh8": woh8, "wol8": wol8,
            "abias": ab_sb, "dvb": dvb, "mk": mk,
        })
    return in_maps


def kernel(hidden_states, past_key, past_value, W_pack_w, o_proj_w):
    nc = _build()
    in_maps = _host_prep(hidden_states, past_key, past_value, W_pack_w, o_proj_w)
    res = run_bass_kernel_spmd(nc, in_maps, list(range(NCORES)))
    out = np.zeros((S, D), np.float64)
    for c in range(NCORES):
        out += res.results[c]["outp"].astype(np.float64)
    return out.astype(np.float32).reshape(B, S, D)

